# revision 23
# baseline (speedup 1.0000x reference)
"""Bass/Trainium2 kernel for nn_GALE_block (dense_transformer, 8 NeuronCores).

Sharding: data-parallel over B (2 groups of 4 cores), sequence-parallel over N
within each group (8192 tokens/core).  Slice-token statistics are combined with
one small AllReduce per group; the tiny slice attention is replicated; the
de-slice + output projection + MLP are fully local.

Structural choices vs a straightforward mapping:
  - LN mean-subtraction folds into column-centered weights host-side
    ((x-m)@W == x@(W-colmean W)); the rstd scale applies post-matmul.  The
    host supplies x token-major (f32, stats/residual) AND feature-major
    (fp8), eliminating all phase-B PE transposes.
  - Wout folds into the slice-attention values host-side (Wvw = Wv@Wout_h,
    Wcvw = Wcv@Wout_h), so de-slice + output projection collapse into one
    256-contraction matmul per tile against OW[hg,c] = mix of attn outputs.
  - All large matmuls run fp8 DoubleRow (2x PE throughput).  Host scales
    weights into fp8 range (SA/SB/SM1/SM2/SOW/SSW); inverse scales fold into
    activation scale factors and the residual-add constants.
  - Elementwise work is spread across Scalar/Vector/GpSimd.
"""

import numpy as np
import ml_dtypes

# problem dims (hardcoded per contest contract)
B, N, C, H, D, G, SC, DC = 2, 32768, 256, 8, 32, 32, 64, 32
NCORES = 8
CPB = 4                      # cores per batch entry
NT_FULL = N // CPB           # tokens per core = 8192
RG = [[0, 1, 2, 3], [4, 5, 6, 7]]
EPS_LN = 1e-5
EPS_SLICE = 1e-5

BF = ml_dtypes.bfloat16
F8 = ml_dtypes.float8_e4m3

SA = 16.0    # Wfx branch fp8 weight scale
SB = 64.0    # Wslice branch fp8 weight scale
SM1 = 16.0   # Wm1 fp8 weight scale
SM2 = 16.0   # Wm2 fp8 weight scale
SOW = 128.0  # attention-value (Wvw/Wcvw) fp8 scale
SSW = 1.0    # slice-weight (sw) fp8 scale (1 = rely on fp8 subnormals)


def _build(NT, flags, sim=False, cut=None):
    """Build the SPMD Bass program for NT tokens/core."""
    proj_bias, m1_bias, wout_bias, m2_bias, cv_bias = flags
    import concourse.bass as bass
    import concourse.bacc as bacc
    import concourse.mybir as mybir
    import concourse.tile as tile
    from contextlib import ExitStack

    f32 = mybir.dt.float32
    bf16 = mybir.dt.bfloat16
    fp8 = mybir.dt.float8e4
    AF = mybir.ActivationFunctionType
    ALU = mybir.AluOpType
    AX = mybir.AxisListType
    DR = mybir.MatmulPerfMode.DoubleRow

    LVL = {"B": 1, "C": 2, "D": 3}.get(cut, 4)
    NTILES = NT // 128
    SCN = 4                       # super-chunks for phase E
    SCT = NTILES // SCN           # tiles per super-chunk

    nc = bacc.Bacc("TRN2", target_bir_lowering=False, debug=False,
                   num_devices=NCORES)

    def din(name, shape, dt=f32):
        return nc.dram_tensor(name, shape, dt, kind="ExternalInput")

    # ---- inputs (host pre-folds weights; see _prep_inputs) ----
    fxq_d = din("fxq", [128, NTILES, C], bf16)    # token-major pretiled bf16
    fxT_d = din("fxT", [128, 2, NT], fp8)         # feature-major fp8
    ctx_d = din("ctx", [H, SC, DC])
    wall_d = din("wall", [128, 2, 512], fp8)      # centered [Wfx'|Wxs] DR fp8
    ball_d = din("ball", [1, 512], bf16)          # fused proj bias row (rare)
    wm1_d = din("wm1", [128, 2, 4 * C], fp8)      # centered g2Wm1*SM1 DR fp8
    bm1_d = din("bm1", [4 * C])                   # fused col bias (fp32)
    wm2_d = din("wm2", [128, 8 * C], fp8)         # Wm2*SM2 DR fp8 [p,(kp a c)]
    bm2_d = din("bm2", [128, C])                  # bm2 replicated (rare)
    bout8_d = din("bout8", [128, 2, C])           # bout*SOW/H replicated (rare)
    wq_d = din("wq", [D, D])
    wk_d = din("wk", [D, D])
    wcq_d = din("wcq", [D, D])
    bcq_d = din("bcq", [D])                       # pre-scaled by D^-0.5
    wck_d = din("wck", [DC, D])
    bck_d = din("bck", [D])
    wvw_d = din("wvw", [D, H, C])                 # Wv @ Wout_h * SOW
    wcvw_d = din("wcvw", [DC, H, C])              # Wcv @ Wout_h * SOW
    bcvw_d = din("bcvw", [1, H, C])               # bcv @ Wout_h * SOW (rare)
    mw_d = din("mw", [128, 1])                    # sigmoid(state_mixing) bcast
    omw_d = din("omw", [128, 1])                  # 1 - mw
    id_bf_d = din("id_bf", [128, 128], bf16)
    id_f8_d = din("id_f8", [128, 128], fp8)
    id_f_d = din("id_f", [128, 128])
    ones_bf_d = din("ones_bf", [1, 128], bf16)
    ones64_d = din("ones64", [1, 64])

    out_d = nc.dram_tensor("out", [128, NTILES, C], f32, kind="ExternalOutput")

    cc_in = nc.dram_tensor("cc_in", [2 * 128, 257], bf16)
    cc_out = nc.dram_tensor("cc_out", [2 * 128, 257], bf16)
    ccd_in = nc.dram_tensor("ccd_in", [1, 1], bf16)
    ccd_out = nc.dram_tensor("ccd_out", [1, 1], bf16)

    ROI = 1.0 / (SOW * SSW)      # o2 unscale const

    with tile.TileContext(nc) as tc, ExitStack() as big:
        wp = big.enter_context(tc.tile_pool(name="wp", bufs=1))
        pers = big.enter_context(tc.tile_pool(name="pers", bufs=1))

        def load(pool, shape, dt, src_ap, tag, eng=None):
            t = pool.tile(shape, dt, tag=tag, name=tag)
            (eng or nc.sync).dma_start(out=t[:], in_=src_ap)
            return t

        wall = load(wp, [128, 2, 512], fp8, wall_d.ap(), "wall")
        ball = load(wp, [1, 512], bf16, ball_d.ap(), "ball")
        id_bf = load(wp, [128, 128], bf16, id_bf_d.ap(), "id_bf")
        id_f8 = load(wp, [128, 128], fp8, id_f8_d.ap(), "id_f8")
        id_f = load(wp, [128, 128], f32, id_f_d.ap(), "id_f")
        ones_bf = load(wp, [1, 128], bf16, ones_bf_d.ap(), "ones_bf")
        ones64 = load(wp, [1, 64], f32, ones64_d.ap(), "ones64")

        def rsqrt_stats(pool, mv_sl, w, outs):
            # outs: list of (dest_slice, mult_const); rstd = rsqrt(var+eps)
            # by Newton from a constant seed (valid for var+eps < 3).
            xe = pool.tile([128, w], f32, tag="rsq_xe", name="rsq_xe")
            y = pool.tile([128, w], f32, tag="rsq_y", name="rsq_y")
            t = pool.tile([128, w], f32, tag="rsq_t", name="rsq_t")
            nc.vector.tensor_scalar_add(out=xe[:], in0=mv_sl[:, :, 1],
                                        scalar1=EPS_LN)
            nc.vector.memset(y[:], 1.0)
            for _ in range(5):  # Newton: y *= 1.5 - 0.5*x*y*y
                nc.vector.tensor_mul(out=t[:], in0=y[:], in1=y[:])
                nc.vector.tensor_mul(out=t[:], in0=t[:], in1=xe[:])
                nc.vector.tensor_scalar_mul(out=t[:], in0=t[:], scalar1=-0.5)
                nc.vector.tensor_scalar_add(out=t[:], in0=t[:], scalar1=1.5)
                nc.vector.tensor_mul(out=y[:], in0=y[:], in1=t[:])
            for dest, mult in outs:
                if mult == 1.0:
                    nc.vector.tensor_copy(out=dest, in_=y[:])
                else:
                    nc.vector.tensor_scalar_mul(out=dest, in0=y[:],
                                                scalar1=mult)

        W = {}

        def late_loads():
            W['wm1'] = load(wp, [128, 2, 4 * C], fp8, wm1_d.ap(), "wm1",
                            nc.scalar)
            W['bm1c'] = [load(wp, [128, 1], f32,
                              bass.AP(bm1_d, 128 * m, [[1, 128], [1, 1]]),
                              f"bm1c{m}", nc.scalar) for m in range(8)]
            W['wm2'] = load(wp, [128, 8 * C], fp8, wm2_d.ap(), "wm2", nc.scalar)
            if m2_bias:
                W['bm2'] = load(wp, [128, C], f32, bm2_d.ap(), "bm2", nc.scalar)
            if wout_bias:
                W['bout8'] = load(wp, [128, 2, C], f32, bout8_d.ap(), "bout8",
                                  nc.scalar)
            W['wq'] = load(wp, [D, D], f32, wq_d.ap(), "wq", nc.scalar)
            W['wk'] = load(wp, [D, D], f32, wk_d.ap(), "wk", nc.scalar)
            W['wcq'] = load(wp, [D, D], f32, wcq_d.ap(), "wcq", nc.scalar)
            W['bcq'] = load(wp, [D, 1], f32,
                            bass.AP(bcq_d, 0, [[1, D], [1, 1]]), "bcq", nc.scalar)
            W['wck'] = load(wp, [DC, D], f32, wck_d.ap(), "wck", nc.scalar)
            W['bck'] = load(wp, [D, 1], f32,
                            bass.AP(bck_d, 0, [[1, D], [1, 1]]), "bck", nc.scalar)
            W['wvw'] = load(wp, [D, H, C], f32, wvw_d.ap(), "wvw", nc.scalar)
            W['wcvw'] = load(wp, [DC, H, C], f32, wcvw_d.ap(), "wcvw", nc.scalar)
            if cv_bias:
                W['bcvw'] = load(wp, [1, H, C], f32, bcvw_d.ap(), "bcvw",
                                 nc.scalar)
            W['mw'] = load(wp, [128, 1], f32, mw_d.ap(), "mw", nc.scalar)
            W['omw'] = load(wp, [128, 1], f32, omw_d.ap(), "omw", nc.scalar)

        # persistent activation buffers
        NQ = max(NTILES // 16, 1)
        fx_q = [pers.tile([128, min(16, NTILES), C], bf16, tag=f"fx_q{q}",
                          name=f"fx_q{q}") for q in range(NQ)]

        def fx_full(i):
            return fx_q[i // 16][:, i % 16, :]

        fxT = pers.tile([128, 2, NT], fp8, tag="fxT")
        swtok = pers.tile([128, NTILES, 256], fp8, tag="swtok")
        swT = pers.tile([128, 2, NT], fp8, tag="swT")
        stp = pers.tile([128, 2, 257], bf16, tag="stp")
        st_head = pers.tile([32, H, 257], bf16, tag="st_head")
        OW = pers.tile([128, 2, C], fp8, tag="OW")
        ctx2 = pers.tile([64, H, DC], f32, tag="ctx2")
        ctxT2 = pers.tile([DC, H, SC], f32, tag="ctxT2")
        ckT2 = pers.tile([D, H, SC], f32, tag="ckT2")
        cvw_sb = pers.tile([SC, H, C], f32, tag="cvw_sb")

        # ============ Phase B: stats, projections, slice weights, st ==========
        with ExitStack() as ph:
            persb = ph.enter_context(tc.tile_pool(name="persb", bufs=1))
            io = ph.enter_context(tc.tile_pool(name="io", bufs=3))
            sb = ph.enter_context(tc.tile_pool(name="sb", bufs=4))
            phb = ExitStack()
            prj = phb.enter_context(tc.tile_pool(name="prj", bufs=4, space="PSUM"))
            stps = phb.enter_context(tc.tile_pool(name="stps", bufs=1, space="PSUM"))

            mvall = persb.tile([128, NTILES, 2], f32, tag="mvall")
            rstdA = persb.tile([128, NTILES], f32, tag="rstdA")
            rstdB = persb.tile([128, NTILES], f32, tag="rstdB")
            fxm4 = persb.tile([128, 4, 2, 257], fp8, tag="fxm4")
            nc.vector.memset(fxm4[:, :, :, 256:257], 1.0)
            st_ps = [stps.tile([128, 257], f32, tag=f"st{m}", name=f"st{m}")
                     for m in range(2)]

            BSZ = min(16, NTILES)

            def stats_batch(lo, hi):
                q = lo // 16
                nc.sync.dma_start(out=fxT[:, :, 128 * lo:128 * hi],
                                  in_=fxT_d.ap()[:, :, 128 * lo:128 * hi])
                nc.sync.dma_start(out=fx_q[q][:, 0:hi - lo, :],
                                  in_=fxq_d.ap()[:, lo:hi, :])
                for i in range(lo, hi):
                    st6 = sb.tile([128, 6], f32, tag="st6")
                    nc.vector.bn_stats(out=st6[:], in_=fx_full(i))
                    nc.vector.bn_aggr(out=mvall[:, i, :], in_=st6[:])
                rsqrt_stats(sb, mvall[:, lo:hi, :], hi - lo,
                            [(rstdA[:, lo:hi], 1.0 / SA),
                             (rstdB[:, lo:hi], 1.0 / SB)])

            def tiles_interleaved():
                for lo in range(0, NTILES, BSZ):
                    stats_batch(lo, min(lo + BSZ, NTILES))
                    if lo == 0:
                        late_loads()
                    yield from range(lo, min(lo + BSZ, NTILES))

            for i in tiles_interleaved():
                pj = prj.tile([128, 512], f32, tag="pj")
                nc.tensor.matmul(pj[:], lhsT=fxT[:, :, 128 * i:128 * (i + 1)],
                                 rhs=wall[:], start=True, stop=not proj_bias,
                                 perf_mode=DR)
                if proj_bias:
                    nc.tensor.matmul(pj[:], lhsT=ones_bf[:1, :], rhs=ball[:1, :],
                                     start=False, stop=True,
                                     skip_group_check=True)
                bsl = (i // 2) % 4
                nc.scalar.activation(out=fxm4[:, bsl, i % 2, 0:256],
                                     in_=pj[:, 0:256],
                                     func=AF.Copy, scale=rstdA[:, i:i + 1])
                u = io.tile([128, 256], f32, tag="u")
                nc.scalar.activation(out=u[:], in_=pj[:, 256:512], func=AF.Exp,
                                     scale=rstdB[:, i:i + 1])
                s8 = sb.tile([128, 8], f32, tag="s8")
                nc.vector.reduce_sum(out=s8[:], in_=u[:].rearrange(
                    "p (h g) -> p h g", h=H), axis=AX.X)
                nc.vector.reciprocal(out=s8[:], in_=s8[:])
                nc.gpsimd.tensor_tensor(
                    out=swtok[:, i, :].rearrange("p (h g) -> p h g", h=H),
                    in0=u[:].rearrange("p (h g) -> p h g", h=H),
                    in1=s8[:].broadcast_to([128, H, G]), op=ALU.mult)
                if i % 2 == 1:
                    for m in range(2):
                        nc.tensor.matmul(
                            st_ps[m][:],
                            lhsT=swtok[:, i - 1:i + 1, 128 * m:128 * (m + 1)],
                            rhs=fxm4[:, bsl, :, :],
                            start=(i == 1), stop=(i == NTILES - 1),
                            perf_mode=DR, skip_group_check=True)

            for m in range(2):
                nc.vector.tensor_copy(out=stp[:, m, :], in_=st_ps[m][:])
            phb.close()

            if LVL == 1:
                stpf = persb.tile([128, 2, 257], f32, tag="stpf")
                nc.vector.tensor_copy(out=stpf[:], in_=stp[:])
                for m in range(2):
                    nc.sync.dma_start(out=out_d.ap()[:, m, 0:256],
                                      in_=stpf[:, m, 0:256])

            # ============ Phase C: AllReduce of slice partials ============
            if LVL >= 2:
                for m in range(2):
                    nc.sync.dma_start(out=cc_in.ap()[128 * m:128 * (m + 1), :],
                                      in_=stp[:, m, :])
                nc.sync.dma_start(out=ccd_in.ap(), in_=stp[0:1, 0, 0:1])
                nc.gpsimd.collective_compute(
                    "AllReduce", ALU.add, ins=[cc_in.ap()], outs=[cc_out.ap()],
                    replica_groups=RG)
                # trailing dummy absorbs the ~20ms completion-poll quantum of
                # the LAST collective in this runtime
                nc.gpsimd.collective_compute(
                    "AllReduce", ALU.add, ins=[ccd_in.ap()], outs=[ccd_out.ap()],
                    replica_groups=RG)

            # --- overlap window: sw -> swT transposes + context prep ---
            if LVL >= 2:
                with ExitStack() as ph2:
                    trp = ph2.enter_context(
                        tc.tile_pool(name="trp", bufs=3, space="PSUM"))
                    cxp = ph2.enter_context(
                        tc.tile_pool(name="cxp", bufs=2, space="PSUM"))
                    ckx = ph2.enter_context(
                        tc.tile_pool(name="ckx", bufs=1, space="PSUM"))
                    for i2 in range(0, NTILES, 2):
                        for m in range(2):
                            sp = trp.tile([128, 256, 2], fp8, tag="swt",
                                          name="swt")
                            for jj in range(2):
                                nc.tensor.transpose(
                                    out=sp[:, 128 * jj:128 * (jj + 1), 0],
                                    in_=swtok[:, i2 + jj,
                                              128 * m:128 * (m + 1)],
                                    identity=id_f8[:])
                            if m == 0:
                                nc.vector.tensor_copy(
                                    out=swT[:, m, 128 * i2:128 * (i2 + 2)],
                                    in_=sp[:, :, 0])
                            else:
                                nc.scalar.activation(
                                    out=swT[:, m, 128 * i2:128 * (i2 + 2)],
                                    in_=sp[:, :, 0], func=AF.Copy)
                    nc.sync.dma_start(out=ctx2[:],
                                      in_=ctx_d.ap().rearrange("h s d -> s h d"))
                    for h in range(H):
                        ctp = cxp.tile([DC, SC], f32, tag="ctp", name="ctp")
                        nc.tensor.transpose(out=ctp[:], in_=ctx2[:, h, :],
                                            identity=id_f[:SC, :SC])
                        nc.vector.tensor_copy(out=ctxT2[:, h, :], in_=ctp[:])
                    ckp = ckx.tile([D, H, SC], f32, tag="ckp")
                    for h in range(H):
                        nc.tensor.matmul(ckp[:, h, :], lhsT=W['wck'][:],
                                         rhs=ctxT2[:, h, :], start=True, stop=True)
                        cvwp = cxp.tile([SC, C], f32, tag="cvwp", name="cvwp")
                        nc.tensor.matmul(cvwp[:], lhsT=ctxT2[:, h, :],
                                         rhs=W['wcvw'][:, h, :],
                                         start=True, stop=not cv_bias)
                        if cv_bias:
                            nc.tensor.matmul(cvwp[:], lhsT=ones64[:1, :],
                                             rhs=W['bcvw'][:1, h, :],
                                             start=False, stop=True)
                        nc.vector.tensor_copy(out=cvw_sb[:, h, :], in_=cvwp[:])
                    nc.scalar.activation(out=ckT2[:], in_=ckp[:],
                                         func=AF.Identity, bias=W['bck'][:])

            if LVL >= 2:
                for h in range(H):
                    nc.sync.dma_start(out=st_head[:, h, :],
                                      in_=cc_out.ap()[32 * h:32 * (h + 1), :])

        if LVL == 2:
            shf = pers.tile([32, H, 257], f32, tag="shf")
            nc.vector.tensor_copy(out=shf[:], in_=st_head[:])
            for h in range(H):
                nc.sync.dma_start(out=out_d.ap()[32 * h:32 * (h + 1), 0, 0:256],
                                  in_=shf[:, h, 0:256])

        # ============ Phase D: slice-token attention (replicated) ============
        if LVL >= 3:
            with ExitStack() as ph:
                ds = ph.enter_context(tc.tile_pool(name="ds", bufs=2))
                rn = ds.tile([32, H], f32, tag="rn")
                nc.vector.tensor_scalar_add(out=rn[:], in0=st_head[:, :, 256],
                                            scalar1=EPS_SLICE)
                nc.vector.reciprocal(out=rn[:], in_=rn[:])
                for h in range(H):
                    nc.vector.tensor_scalar_mul(out=st_head[:, h, 0:256],
                                                in0=st_head[:, h, 0:256],
                                                scalar1=rn[:, h:h + 1])
                with tc.tile_pool(name="dpA", bufs=1, space="PSUM") as dpA:
                    stT_ps = dpA.tile([32, 256], bf16, tag="stT")
                    for h in range(H):
                        nc.tensor.transpose(
                            out=stT_ps[:, 32 * h:32 * (h + 1)],
                            in_=st_head[:, h, 32 * h:32 * (h + 1)],
                            identity=id_bf[:32, :32])
                    stT2 = ds.tile([32, 256], f32, tag="stT2")
                    nc.vector.tensor_copy(out=stT2[:], in_=stT_ps[:])
                    qkc = dpA.tile([32, 3, 256], f32, tag="qkc")
                    nc.tensor.matmul(qkc[:, 0, :], lhsT=W['wq'][:], rhs=stT2[:],
                                     start=True, stop=True)
                    nc.tensor.matmul(qkc[:, 1, :], lhsT=W['wk'][:], rhs=stT2[:],
                                     start=True, stop=True)
                    nc.tensor.matmul(qkc[:, 2, :], lhsT=W['wcq'][:], rhs=stT2[:],
                                     start=True, stop=True)
                    vw_ps = dpA.tile([32, H, 256], f32, tag="vw_ps")
                    for h in range(H):
                        nc.tensor.matmul(vw_ps[:, h, :],
                                         lhsT=stT2[:, 32 * h:32 * (h + 1)],
                                         rhs=W['wvw'][:, h, :],
                                         start=True, stop=True)
                    qT2 = ds.tile([32, 256], f32, tag="qT2")
                    nc.scalar.activation(out=qT2[:], in_=qkc[:, 0, :],
                                         func=AF.Copy, scale=float(D) ** -0.5)
                    kT2 = ds.tile([32, 256], f32, tag="kT2")
                    nc.vector.tensor_copy(out=kT2[:], in_=qkc[:, 1, :])
                    cqT2 = ds.tile([32, 256], f32, tag="cqT2")
                    nc.scalar.activation(out=cqT2[:], in_=qkc[:, 2, :],
                                         func=AF.Identity, bias=W['bcq'][:],
                                         scale=float(D) ** -0.5)
                    vw2 = ds.tile([32, H, 256], f32, tag="vw2")
                    nc.vector.tensor_copy(out=vw2[:], in_=vw_ps[:])
                dp = ph.enter_context(tc.tile_pool(name="dpB", bufs=1,
                                                   space="PSUM"))

                def softmax_rows(logits_ps, width, nheads, tag):
                    uu = ds.tile([32, nheads * width], f32, tag=tag + "u")
                    nc.scalar.activation(out=uu[:], in_=logits_ps[:], func=AF.Exp)
                    ss = ds.tile([32, nheads], f32, tag=tag + "s")
                    nc.vector.reduce_sum(out=ss[:], in_=uu[:].rearrange(
                        "p (h w) -> p h w", h=nheads), axis=AX.X)
                    nc.vector.reciprocal(out=ss[:], in_=ss[:])
                    nc.vector.tensor_tensor(
                        out=uu[:].rearrange("p (h w) -> p h w", h=nheads),
                        in0=uu[:].rearrange("p (h w) -> p h w", h=nheads),
                        in1=ss[:].broadcast_to([32, nheads, width]), op=ALU.mult)
                    return uu

                slp = dp.tile([32, 256], f32, tag="p32")
                for h in range(H):
                    sl = slice(32 * h, 32 * (h + 1))
                    nc.tensor.matmul(slp[:, sl], lhsT=qT2[:, sl], rhs=kT2[:, sl],
                                     start=True, stop=True)
                sattn = softmax_rows(slp, G, H, "sa")
                saT_ps = dp.tile([32, 256], f32, tag="p32", name="saT_ps")
                for h in range(H):
                    sl = slice(32 * h, 32 * (h + 1))
                    nc.tensor.transpose(out=saT_ps[:, sl], in_=sattn[:, sl],
                                        identity=id_f[:32, :32])
                saT = ds.tile([32, 256], f32, tag="saT")
                nc.vector.tensor_copy(out=saT[:], in_=saT_ps[:])

                clp = dp.tile([32, 512], f32, tag="p64")
                for h in range(H):
                    nc.tensor.matmul(clp[:, 64 * h:64 * (h + 1)],
                                     lhsT=cqT2[:, 32 * h:32 * (h + 1)],
                                     rhs=ckT2[:, h, :], start=True, stop=True)
                cattn = softmax_rows(clp, SC, H, "ca")
                caT_ps = dp.tile([64, 256], f32, tag="p64", name="caT_ps")
                for h in range(H):
                    nc.tensor.transpose(out=caT_ps[:, 32 * h:32 * (h + 1)],
                                        in_=cattn[:, 64 * h:64 * (h + 1)],
                                        identity=id_f[:32, :32])
                caT = ds.tile([64, 256], f32, tag="caT")
                nc.vector.tensor_copy(out=caT[:], in_=caT_ps[:])

                self_ps = dp.tile([128, 2, 256], f32, tag="self_ps")
                cross_ps = dp.tile([128, 2, 256], f32, tag="cross_ps")
                for h in range(H):
                    sl = slice(32 * h, 32 * (h + 1))
                    r0 = 32 * (h % 4)
                    nc.tensor.matmul(self_ps[r0:r0 + 32, h // 4, :],
                                     lhsT=saT[:, sl], rhs=vw2[:, h, :],
                                     start=True, stop=True,
                                     tile_position=(0, r0))
                    nc.tensor.matmul(cross_ps[r0:r0 + 32, h // 4, :],
                                     lhsT=caT[:, sl], rhs=cvw_sb[:, h, :],
                                     start=True, stop=True,
                                     tile_position=(0, r0))
                gtmp = ds.tile([128, 2, 256], f32, tag="gtmp")
                nc.vector.tensor_scalar_mul(out=gtmp[:], in0=self_ps[:],
                                            scalar1=W['mw'][:])
                if wout_bias:
                    nc.vector.tensor_add(out=gtmp[:], in0=gtmp[:],
                                         in1=W['bout8'][:])
                nc.vector.scalar_tensor_tensor(out=OW[:], in0=cross_ps[:],
                                               scalar=W['omw'][:], in1=gtmp[:],
                                               op0=ALU.mult, op1=ALU.add)

        if LVL == 3:
            owf = pers.tile([128, 2, 256], f32, tag="owf")
            nc.vector.tensor_copy(out=owf[:], in_=OW[:])
            for m in range(2):
                nc.sync.dma_start(out=out_d.ap()[:, m, :], in_=owf[:, m, :])

        # ============ Phase E: de-slice+Wout, LN2, MLP (merged pipeline) =====
        if LVL >= 4:
            with ExitStack() as ph:
                eio = ph.enter_context(tc.tile_pool(name="eio", bufs=3))
                esb = ph.enter_context(tc.tile_pool(name="esb", bufs=4))
                perse = ph.enter_context(tc.tile_pool(name="perse", bufs=1))
                o2p = ph.enter_context(tc.tile_pool(name="o2p", bufs=2,
                                                    space="PSUM"))
                ztp2 = ph.enter_context(tc.tile_pool(name="ztp2", bufs=2,
                                                     space="PSUM"))
                m1p = ph.enter_context(tc.tile_pool(name="m1p", bufs=2,
                                                    space="PSUM"))
                smp = ph.enter_context(tc.tile_pool(name="smp", bufs=2,
                                                    space="PSUM"))
                mv2 = perse.tile([128, NTILES, 2], f32, tag="mv2")
                rstd2 = perse.tile([128, NTILES], f32, tag="rstd2")
                wm2_4d = W['wm2'][:].rearrange("p (kp a c) -> p kp a c",
                                               kp=4, a=2)

                for scc in range(SCN):
                    tlo, thi = SCT * scc, SCT * (scc + 1)
                    for i in range(tlo, thi):
                        o2 = o2p.tile([128, C], f32, tag="o2", name="o2")
                        nc.tensor.matmul(
                            o2[:], lhsT=swT[:, :, 128 * i:128 * (i + 1)],
                            rhs=OW[:], start=True, stop=True, perf_mode=DR)
                        nc.vector.scalar_tensor_tensor(
                            out=fx_full(i), in0=o2[:], scalar=ROI,
                            in1=fx_full(i), op0=ALU.mult, op1=ALU.add)
                        st6 = esb.tile([128, 6], f32, tag="st6")
                        nc.vector.bn_stats(out=st6[:], in_=fx_full(i))
                        nc.vector.bn_aggr(out=mv2[:, i, :], in_=st6[:])
                    rsqrt_stats(esb, mv2[:, tlo:thi, :], thi - tlo,
                                [(rstd2[:, tlo:thi], 1.0)])
                    for ci in range(4 * scc, 4 * (scc + 1)):
                        z2T = eio.tile([128, 2, 512], fp8, tag="z2T")
                        for j2 in range(2):
                            zt_ps = ztp2.tile([128, 2, 2, 128], bf16, tag="z2t")
                            for jj in range(2):
                                i = 4 * ci + 2 * j2 + jj
                                z2 = esb.tile([128, C], bf16, tag="z2")
                                nc.gpsimd.tensor_tensor(
                                    out=z2[:], in0=fx_full(i),
                                    in1=rstd2[:, i:i + 1].broadcast_to([128, C]),
                                    op=ALU.mult)
                                for k in range(2):
                                    nc.tensor.transpose(
                                        out=zt_ps[:, jj, k, :],
                                        in_=z2[:, 128 * k:128 * (k + 1)],
                                        identity=id_bf[:])
                            nc.vector.tensor_copy(
                                out=z2T[:, :, 256 * j2:256 * (j2 + 1)]
                                .rearrange("p k (a t) -> p k a t", a=2),
                                in_=zt_ps[:].rearrange("p a k t -> p k a t"))
                        m1T = eio.tile([128, 8, 512], fp8, tag="m1T")
                        for mt in range(8):
                            mp = m1p.tile([128, 512], f32, tag="m1", name="m1")
                            nc.tensor.matmul(
                                mp[:],
                                lhsT=W['wm1'][:, :, 128 * mt:128 * (mt + 1)],
                                rhs=z2T[:], start=True, stop=True,
                                perf_mode=DR)
                            if not m1_bias:
                                nc.scalar.activation(
                                    out=m1T[:, mt, :], in_=mp[:],
                                    func=(AF.Identity if sim else AF.Gelu),
                                    scale=1.0 / SM1)
                            else:
                                nc.scalar.activation(
                                    out=m1T[:, mt, :], in_=mp[:],
                                    func=(AF.Identity if sim else AF.Gelu),
                                    scale=1.0 / SM1, bias=W['bm1c'][mt][:])
                        o_t = eio.tile([128, 4, C], f32, tag="ot")
                        for j in range(4):
                            i = 4 * ci + j
                            m2ps = smp.tile([128, C], f32, tag="m2", name="m2ps")
                            for kp in range(4):
                                nc.tensor.matmul(
                                    m2ps[:],
                                    lhsT=m1T[:, 2 * kp:2 * kp + 2,
                                             128 * j:128 * (j + 1)],
                                    rhs=wm2_4d[:, kp, :, :],
                                    start=(kp == 0), stop=(kp == 3),
                                    perf_mode=DR)
                            nc.vector.scalar_tensor_tensor(
                                out=o_t[:, j, :], in0=m2ps[:], scalar=1.0 / SM2,
                                in1=fx_full(i), op0=ALU.mult, op1=ALU.add)
                            if m2_bias:
                                nc.vector.tensor_add(out=o_t[:, j, :],
                                                     in0=o_t[:, j, :],
                                                     in1=W['bm2'][:])
                        nc.sync.dma_start(
                            out=out_d.ap()[:, 4 * ci:4 * (ci + 1), :],
                            in_=o_t[:])

    nc.compile()
    return nc


def _prep_inputs(NT, inputs):
    """Host-side weight folding + per-core input maps."""
    f = lambda x: np.asarray(x, np.float32)
    g1 = f(inputs["ln1_g"]); b1 = f(inputs["ln1_b"])
    g2 = f(inputs["ln2_g"]); b2 = f(inputs["ln2_b"])
    Wfx = f(inputs["Wfx"]); bfx = f(inputs["bfx"])
    Wx = f(inputs["Wx"]); bx = f(inputs["bx"])
    Wslice = f(inputs["Wslice"]); bslice = f(inputs["bslice"])
    temp = f(inputs["temperature"]).reshape(H)
    Wm1 = f(inputs["Wm1"]); bm1 = f(inputs["bm1"])
    Wm2 = f(inputs["Wm2"]); bm2 = f(inputs["bm2"])
    Wout = f(inputs["Wout"]); bout = f(inputs["bout"])
    Wq = f(inputs["Wq"]); Wk = f(inputs["Wk"]); Wv = f(inputs["Wv"])
    Wcq = f(inputs["Wcq"]); bcq = f(inputs["bcq"])
    Wck = f(inputs["Wck"]); bck = f(inputs["bck"])
    Wcv = f(inputs["Wcv"]); bcv = f(inputs["bcv"])
    scale = float(D) ** -0.5

    # block-diag Wslice scaled by 1/temperature
    Wbd = np.zeros((H * D, H * G), np.float32)
    for h in range(H):
        Wbd[h * D:(h + 1) * D, h * G:(h + 1) * G] = Wslice / temp[h]
    bslice_rep = np.concatenate([bslice / temp[h] for h in range(H)])

    # column-centered, fp8-scaled fused projection weights:
    # (x - mean(x)) @ W == x @ (W - colmean(W))
    Wfxp = g1[:, None] * Wfx
    Wxs = (g1[:, None] * Wx) @ Wbd
    Wfxp_c = (Wfxp - Wfxp.mean(0, keepdims=True)) * SA
    Wxs_c = (Wxs - Wxs.mean(0, keepdims=True)) * SB
    wall_full = np.concatenate([Wfxp_c, Wxs_c], axis=1)       # [C, 512]
    wall_dr = np.ascontiguousarray(
        wall_full.reshape(2, 128, 512).transpose(1, 0, 2)).astype(F8)
    ball_f = np.concatenate([(b1 @ Wfx + bfx) * SA,
                             ((b1 @ Wx + bx) @ Wbd + bslice_rep) * SB])
    ball = ball_f[None, :].astype(BF)
    proj_bias = bool(np.any(ball_f != 0.0))

    # MLP weights: column-centered g2*Wm1, fp8 DR layouts
    W1 = g2[:, None] * Wm1
    W1_c = (W1 - W1.mean(0, keepdims=True)) * SM1              # [C, 4C]
    wm1_dr = np.ascontiguousarray(
        W1_c.reshape(2, 128, 4 * C).transpose(1, 0, 2)).astype(F8)
    bm1p = (b2 @ Wm1 + bm1).astype(np.float32)
    m1_bias = bool(np.any(bm1p != 0.0))
    wm2_dr = np.ascontiguousarray(
        (Wm2 * SM2).reshape(4, 2, 128, C).transpose(2, 0, 1, 3)
        .reshape(128, 8 * C)).astype(F8)
    bm2_rep = np.ascontiguousarray(
        np.broadcast_to(bm2[None, :], (128, C)), np.float32)
    m2_bias = bool(np.any(bm2 != 0.0))

    # Wout folded into attention values (scaled by SOW for fp8 range)
    Wvw = np.stack([Wv @ Wout[32 * h:32 * (h + 1), :] for h in range(H)],
                   1) * SOW
    Wcvw = np.stack([Wcv @ Wout[32 * h:32 * (h + 1), :] for h in range(H)],
                    1) * SOW
    bcvw = np.stack([bcv @ Wout[32 * h:32 * (h + 1), :] for h in range(H)],
                    0) * SOW
    cv_bias = bool(np.any(bcv != 0.0))
    bout8 = np.ascontiguousarray(
        np.broadcast_to(bout[None, None, :] * (SOW / H), (128, 2, C)),
        np.float32)
    wout_bias = bool(np.any(bout != 0.0))

    mwv = float(1.0 / (1.0 + np.exp(-f(inputs["state_mixing"]))))
    mw = np.full((128, 1), mwv, np.float32)
    omw = np.full((128, 1), 1.0 - mwv, np.float32)

    id_f = np.eye(128, dtype=np.float32)

    common = dict(
        wall=wall_dr, ball=ball, wm1=wm1_dr, bm1=bm1p, wm2=wm2_dr,
        bm2=bm2_rep, bout8=bout8,
        wq=Wq, wk=Wk, wcq=Wcq, bcq=(bcq * scale).astype(np.float32),
        wck=Wck, bck=bck,
        wvw=np.ascontiguousarray(Wvw, np.float32),
        wcvw=np.ascontiguousarray(Wcvw, np.float32),
        bcvw=np.ascontiguousarray(bcvw[None, :, :], np.float32),
        mw=mw, omw=omw,
        id_bf=id_f.astype(BF), id_f8=id_f.astype(F8), id_f=id_f,
        ones_bf=np.ones((1, 128), BF), ones64=np.ones((1, 64), np.float32),
    )

    fx = f(inputs["fx"])
    ctxt = f(inputs["context"])
    in_maps = []
    for core in range(NCORES):
        b, s = core // CPB, core % CPB
        x = fx[b, s * NT:(s + 1) * NT, :]                      # [NT, C]
        m = dict(common)
        m["fxq"] = np.ascontiguousarray(
            x.reshape(NT // 128, 128, C).transpose(1, 0, 2)).astype(BF)
        m["fxT"] = np.ascontiguousarray(
            x.T.reshape(2, 128, NT).transpose(1, 0, 2)).astype(F8)
        m["ctx"] = np.ascontiguousarray(ctxt[b])
        in_maps.append(m)
    return in_maps, (proj_bias, m1_bias, wout_bias, m2_bias, cv_bias)


_CACHE = {}


def _get_compiled(NT, flags):
    key = (NT,) + flags
    if key not in _CACHE:
        _CACHE[key] = _build(NT, flags)
    return _CACHE[key]


def kernel(**inputs):
    from concourse.bass_utils import run_bass_kernel_spmd
    NT = NT_FULL
    in_maps, flags = _prep_inputs(NT, inputs)
    nc = _get_compiled(NT, flags)
    res = run_bass_kernel_spmd(nc, in_maps, list(range(NCORES)))
    out = np.empty((B, N, C), np.float32)
    for core in range(NCORES):
        b, s = core // CPB, core % CPB
        out[b, s * NT:(s + 1) * NT, :] = (
            res.results[core]["out"].transpose(1, 0, 2).reshape(NT, C))
    return out


# revision 24
# speedup vs baseline: 1.2060x; 1.2060x over previous
"""Bass/Trainium2 kernel for nn_GALE_block (dense_transformer, 8 NeuronCores).

Sharding: data-parallel over B (2 groups of 4 cores), sequence-parallel over N
within each group (8192 tokens/core).  Slice-token statistics are combined with
one small AllReduce per group; the tiny slice attention is replicated; the
de-slice + output projection + MLP are fully local.

Structural choices vs a straightforward mapping:
  - LN mean-subtraction folds into column-centered weights host-side
    ((x-m)@W == x@(W-colmean W)); the rstd scale applies post-matmul.  The
    host supplies x token-major (f32, stats/residual) AND feature-major
    (fp8), eliminating all phase-B PE transposes.
  - Wout folds into the slice-attention values host-side (Wvw = Wv@Wout_h,
    Wcvw = Wcv@Wout_h), so de-slice + output projection collapse into one
    256-contraction matmul per tile against OW[hg,c] = mix of attn outputs.
  - All large matmuls run fp8 DoubleRow (2x PE throughput).  Host scales
    weights into fp8 range (SA/SB/SM1/SM2/SOW/SSW); inverse scales fold into
    activation scale factors and the residual-add constants.
  - Elementwise work is spread across Scalar/Vector/GpSimd.
"""

import numpy as np
import ml_dtypes

# problem dims (hardcoded per contest contract)
B, N, C, H, D, G, SC, DC = 2, 32768, 256, 8, 32, 32, 64, 32
NCORES = 8
CPB = 4                      # cores per batch entry
NT_FULL = N // CPB           # tokens per core = 8192
RG = [[0, 1, 2, 3], [4, 5, 6, 7]]
EPS_LN = 1e-5
EPS_SLICE = 1e-5

BF = ml_dtypes.bfloat16
F8 = ml_dtypes.float8_e4m3

SA = 16.0    # Wfx branch fp8 weight scale
SB = 64.0    # Wslice branch fp8 weight scale
SM1 = 16.0   # Wm1 fp8 weight scale
SM2 = 16.0   # Wm2 fp8 weight scale
SOW = 128.0  # attention-value (Wvw/Wcvw) fp8 scale
SSW = 1.0    # slice-weight (sw) fp8 scale (1 = rely on fp8 subnormals)


def _build(NT, flags, sim=False, cut=None):
    """Build the SPMD Bass program for NT tokens/core."""
    proj_bias, m1_bias, wout_bias, m2_bias, cv_bias = flags
    import concourse.bass as bass
    import concourse.bacc as bacc
    import concourse.mybir as mybir
    import concourse.tile as tile
    from contextlib import ExitStack

    f32 = mybir.dt.float32
    bf16 = mybir.dt.bfloat16
    fp8 = mybir.dt.float8e4
    AF = mybir.ActivationFunctionType
    ALU = mybir.AluOpType
    AX = mybir.AxisListType
    DR = mybir.MatmulPerfMode.DoubleRow

    LVL = {"B": 1, "C": 2, "D": 3}.get(cut, 4)
    NTILES = NT // 128
    SCN = 4                       # super-chunks for phase E
    SCT = NTILES // SCN           # tiles per super-chunk

    nc = bacc.Bacc("TRN2", target_bir_lowering=False, debug=False,
                   num_devices=NCORES)

    def din(name, shape, dt=f32):
        return nc.dram_tensor(name, shape, dt, kind="ExternalInput")

    # ---- inputs (host pre-folds weights; see _prep_inputs) ----
    fxq_d = din("fxq", [128, NTILES, C])          # token-major pretiled f32
    fxT_d = din("fxT", [128, 2, NT], fp8)         # feature-major fp8
    ctx_d = din("ctx", [H, SC, DC])
    wall_d = din("wall", [128, 2, 512], fp8)      # centered [Wfx'|Wxs] DR fp8
    ball_d = din("ball", [1, 512], bf16)          # fused proj bias row (rare)
    wm1_d = din("wm1", [128, 2, 4 * C], fp8)      # centered g2Wm1*SM1 DR fp8
    bm1_d = din("bm1", [4 * C])                   # fused col bias (fp32)
    wm2_d = din("wm2", [128, 8 * C], fp8)         # Wm2*SM2 DR fp8 [p,(kp a c)]
    bm2_d = din("bm2", [128, C])                  # bm2 replicated (rare)
    bout8_d = din("bout8", [128, 2, C])           # bout*SOW/H replicated (rare)
    wq_d = din("wq", [D, D])
    wk_d = din("wk", [D, D])
    wcq_d = din("wcq", [D, D])
    bcq_d = din("bcq", [D])                       # pre-scaled by D^-0.5
    wck_d = din("wck", [DC, D])
    bck_d = din("bck", [D])
    wvw_d = din("wvw", [D, H, C])                 # Wv @ Wout_h * SOW
    wcvw_d = din("wcvw", [DC, H, C])              # Wcv @ Wout_h * SOW
    bcvw_d = din("bcvw", [1, H, C])               # bcv @ Wout_h * SOW (rare)
    mw_d = din("mw", [128, 1])                    # sigmoid(state_mixing) bcast
    omw_d = din("omw", [128, 1])                  # 1 - mw
    id_bf_d = din("id_bf", [128, 128], bf16)
    id_f8_d = din("id_f8", [128, 128], fp8)
    id_f_d = din("id_f", [128, 128])
    ones_bf_d = din("ones_bf", [1, 128], bf16)
    ones64_d = din("ones64", [1, 64])

    out_d = nc.dram_tensor("out", [128, NTILES, C], f32, kind="ExternalOutput")

    cc_in = nc.dram_tensor("cc_in", [2 * 128, 257], bf16)
    cc_out = nc.dram_tensor("cc_out", [2 * 128, 257], bf16)
    ccd_in = nc.dram_tensor("ccd_in", [1, 1], bf16)
    ccd_out = nc.dram_tensor("ccd_out", [1, 1], bf16)

    ROI = 1.0 / (SOW * SSW)      # o2 unscale const

    with tile.TileContext(nc) as tc, ExitStack() as big:
        wp = big.enter_context(tc.tile_pool(name="wp", bufs=1))
        pers = big.enter_context(tc.tile_pool(name="pers", bufs=1))

        def load(pool, shape, dt, src_ap, tag, eng=None):
            t = pool.tile(shape, dt, tag=tag, name=tag)
            (eng or nc.sync).dma_start(out=t[:], in_=src_ap)
            return t

        wall = load(wp, [128, 2, 512], fp8, wall_d.ap(), "wall")
        ball = load(wp, [1, 512], bf16, ball_d.ap(), "ball")
        id_bf = load(wp, [128, 128], bf16, id_bf_d.ap(), "id_bf")
        id_f8 = load(wp, [128, 128], fp8, id_f8_d.ap(), "id_f8")
        id_f = load(wp, [128, 128], f32, id_f_d.ap(), "id_f")
        ones_bf = load(wp, [1, 128], bf16, ones_bf_d.ap(), "ones_bf")
        ones64 = load(wp, [1, 64], f32, ones64_d.ap(), "ones64")

        def rsqrt_stats(pool, mv_sl, w, outs):
            # outs: list of (dest_slice, mult_const); rstd = rsqrt(var+eps)
            # by Newton from a constant seed (valid for var+eps < 3).
            xe = pool.tile([128, w], f32, tag="rsq_xe", name="rsq_xe")
            y = pool.tile([128, w], f32, tag="rsq_y", name="rsq_y")
            t = pool.tile([128, w], f32, tag="rsq_t", name="rsq_t")
            nc.vector.tensor_scalar_add(out=xe[:], in0=mv_sl[:, :, 1],
                                        scalar1=EPS_LN)
            nc.vector.memset(y[:], 1.0)
            for _ in range(5):  # Newton: y *= 1.5 - 0.5*x*y*y
                nc.vector.tensor_mul(out=t[:], in0=y[:], in1=y[:])
                nc.vector.tensor_mul(out=t[:], in0=t[:], in1=xe[:])
                nc.vector.tensor_scalar_mul(out=t[:], in0=t[:], scalar1=-0.5)
                nc.vector.tensor_scalar_add(out=t[:], in0=t[:], scalar1=1.5)
                nc.vector.tensor_mul(out=y[:], in0=y[:], in1=t[:])
            for dest, mult in outs:
                if mult == 1.0:
                    nc.vector.tensor_copy(out=dest, in_=y[:])
                else:
                    nc.vector.tensor_scalar_mul(out=dest, in0=y[:],
                                                scalar1=mult)

        W = {}

        def late_loads():
            W['wm1'] = load(wp, [128, 2, 4 * C], fp8, wm1_d.ap(), "wm1",
                            nc.scalar)
            W['bm1c'] = [load(wp, [128, 1], f32,
                              bass.AP(bm1_d, 128 * m, [[1, 128], [1, 1]]),
                              f"bm1c{m}", nc.scalar) for m in range(8)]
            W['wm2'] = load(wp, [128, 8 * C], fp8, wm2_d.ap(), "wm2", nc.scalar)
            if m2_bias:
                W['bm2'] = load(wp, [128, C], f32, bm2_d.ap(), "bm2", nc.scalar)
            if wout_bias:
                W['bout8'] = load(wp, [128, 2, C], f32, bout8_d.ap(), "bout8",
                                  nc.scalar)
            W['wq'] = load(wp, [D, D], f32, wq_d.ap(), "wq", nc.scalar)
            W['wk'] = load(wp, [D, D], f32, wk_d.ap(), "wk", nc.scalar)
            W['wcq'] = load(wp, [D, D], f32, wcq_d.ap(), "wcq", nc.scalar)
            W['bcq'] = load(wp, [D, 1], f32,
                            bass.AP(bcq_d, 0, [[1, D], [1, 1]]), "bcq", nc.scalar)
            W['wck'] = load(wp, [DC, D], f32, wck_d.ap(), "wck", nc.scalar)
            W['bck'] = load(wp, [D, 1], f32,
                            bass.AP(bck_d, 0, [[1, D], [1, 1]]), "bck", nc.scalar)
            W['wvw'] = load(wp, [D, H, C], f32, wvw_d.ap(), "wvw", nc.scalar)
            W['wcvw'] = load(wp, [DC, H, C], f32, wcvw_d.ap(), "wcvw", nc.scalar)
            if cv_bias:
                W['bcvw'] = load(wp, [1, H, C], f32, bcvw_d.ap(), "bcvw",
                                 nc.scalar)
            W['mw'] = load(wp, [128, 1], f32, mw_d.ap(), "mw", nc.scalar)
            W['omw'] = load(wp, [128, 1], f32, omw_d.ap(), "omw", nc.scalar)

        # persistent activation buffers
        NQ = max(NTILES // 16, 1)
        fx_q = [pers.tile([128, min(16, NTILES), C], f32, tag=f"fx_q{q}",
                          name=f"fx_q{q}") for q in range(NQ)]

        def fx_full(i):
            return fx_q[i // 16][:, i % 16, :]

        fxT = pers.tile([128, 2, NT], fp8, tag="fxT")
        swtok = pers.tile([128, NTILES, 256], fp8, tag="swtok")
        swT = pers.tile([128, 2, NT], fp8, tag="swT")
        stp = pers.tile([128, 2, 257], bf16, tag="stp")
        st_head = pers.tile([32, H, 257], bf16, tag="st_head")
        OW = pers.tile([128, 2, C], fp8, tag="OW")
        ctx2 = pers.tile([64, H, DC], f32, tag="ctx2")
        ctxT2 = pers.tile([DC, H, SC], f32, tag="ctxT2")
        ckT2 = pers.tile([D, H, SC], f32, tag="ckT2")
        cvw_sb = pers.tile([SC, H, C], f32, tag="cvw_sb")

        # ============ Phase B: stats, projections, slice weights, st ==========
        with ExitStack() as ph:
            persb = ph.enter_context(tc.tile_pool(name="persb", bufs=1))
            io = ph.enter_context(tc.tile_pool(name="io", bufs=3))
            sb = ph.enter_context(tc.tile_pool(name="sb", bufs=4))
            phb = ExitStack()
            prj = phb.enter_context(tc.tile_pool(name="prj", bufs=4, space="PSUM"))
            stps = phb.enter_context(tc.tile_pool(name="stps", bufs=1, space="PSUM"))

            mvall = persb.tile([128, NTILES, 2], f32, tag="mvall")
            rstdA = persb.tile([128, NTILES], f32, tag="rstdA")
            rstdB = persb.tile([128, NTILES], f32, tag="rstdB")
            fxm4 = persb.tile([128, 4, 2, 257], fp8, tag="fxm4")
            nc.vector.memset(fxm4[:, :, :, 256:257], 1.0)
            st_ps = [stps.tile([128, 257], f32, tag=f"st{m}", name=f"st{m}")
                     for m in range(2)]

            BSZ = min(16, NTILES)

            def stats_batch(lo, hi):
                q = lo // 16
                nc.sync.dma_start(out=fx_q[q][:, 0:hi - lo, :],
                                  in_=fxq_d.ap()[:, lo:hi, :])
                nc.sync.dma_start(out=fxT[:, :, 128 * lo:128 * hi],
                                  in_=fxT_d.ap()[:, :, 128 * lo:128 * hi])
                for i in range(lo, hi):
                    st6 = sb.tile([128, 6], f32, tag="st6")
                    nc.vector.bn_stats(out=st6[:], in_=fx_full(i))
                    nc.vector.bn_aggr(out=mvall[:, i, :], in_=st6[:])
                rsqrt_stats(sb, mvall[:, lo:hi, :], hi - lo,
                            [(rstdA[:, lo:hi], 1.0 / SA),
                             (rstdB[:, lo:hi], 1.0 / SB)])

            def tiles_interleaved():
                for lo in range(0, NTILES, BSZ):
                    stats_batch(lo, min(lo + BSZ, NTILES))
                    if lo == 0:
                        late_loads()
                    yield from range(lo, min(lo + BSZ, NTILES))

            for i in tiles_interleaved():
                pj = prj.tile([128, 512], f32, tag="pj")
                nc.tensor.matmul(pj[:], lhsT=fxT[:, :, 128 * i:128 * (i + 1)],
                                 rhs=wall[:], start=True, stop=not proj_bias,
                                 perf_mode=DR)
                if proj_bias:
                    nc.tensor.matmul(pj[:], lhsT=ones_bf[:1, :], rhs=ball[:1, :],
                                     start=False, stop=True,
                                     skip_group_check=True)
                bsl = (i // 2) % 4
                nc.scalar.activation(out=fxm4[:, bsl, i % 2, 0:256],
                                     in_=pj[:, 0:256],
                                     func=AF.Copy, scale=rstdA[:, i:i + 1])
                u = io.tile([128, 256], f32, tag="u")
                nc.scalar.activation(out=u[:], in_=pj[:, 256:512], func=AF.Exp,
                                     scale=rstdB[:, i:i + 1])
                s8 = sb.tile([128, 8], f32, tag="s8")
                nc.vector.reduce_sum(out=s8[:], in_=u[:].rearrange(
                    "p (h g) -> p h g", h=H), axis=AX.X)
                nc.vector.reciprocal(out=s8[:], in_=s8[:])
                nc.gpsimd.tensor_tensor(
                    out=swtok[:, i, :].rearrange("p (h g) -> p h g", h=H),
                    in0=u[:].rearrange("p (h g) -> p h g", h=H),
                    in1=s8[:].broadcast_to([128, H, G]), op=ALU.mult)
                if i % 2 == 1:
                    for m in range(2):
                        nc.tensor.matmul(
                            st_ps[m][:],
                            lhsT=swtok[:, i - 1:i + 1, 128 * m:128 * (m + 1)],
                            rhs=fxm4[:, bsl, :, :],
                            start=(i == 1), stop=(i == NTILES - 1),
                            perf_mode=DR, skip_group_check=True)

            for m in range(2):
                nc.vector.tensor_copy(out=stp[:, m, :], in_=st_ps[m][:])
            phb.close()

            if LVL == 1:
                stpf = persb.tile([128, 2, 257], f32, tag="stpf")
                nc.vector.tensor_copy(out=stpf[:], in_=stp[:])
                for m in range(2):
                    nc.sync.dma_start(out=out_d.ap()[:, m, 0:256],
                                      in_=stpf[:, m, 0:256])

            # ============ Phase C: AllReduce of slice partials ============
            if LVL >= 2:
                for m in range(2):
                    nc.sync.dma_start(out=cc_in.ap()[128 * m:128 * (m + 1), :],
                                      in_=stp[:, m, :])
                nc.sync.dma_start(out=ccd_in.ap(), in_=stp[0:1, 0, 0:1])
                nc.gpsimd.collective_compute(
                    "AllReduce", ALU.add, ins=[cc_in.ap()], outs=[cc_out.ap()],
                    replica_groups=RG)
                # trailing dummy absorbs the ~20ms completion-poll quantum of
                # the LAST collective in this runtime
                nc.gpsimd.collective_compute(
                    "AllReduce", ALU.add, ins=[ccd_in.ap()], outs=[ccd_out.ap()],
                    replica_groups=RG)

            # --- overlap window: sw -> swT transposes + context prep ---
            if LVL >= 2:
                with ExitStack() as ph2:
                    trp = ph2.enter_context(
                        tc.tile_pool(name="trp", bufs=3, space="PSUM"))
                    cxp = ph2.enter_context(
                        tc.tile_pool(name="cxp", bufs=2, space="PSUM"))
                    ckx = ph2.enter_context(
                        tc.tile_pool(name="ckx", bufs=1, space="PSUM"))
                    for i in range(NTILES):
                        for m in range(2):
                            sp = trp.tile([128, 128, 2], fp8, tag="swt",
                                          name="swt")
                            nc.tensor.transpose(
                                out=sp[:, :, 0],
                                in_=swtok[:, i, 128 * m:128 * (m + 1)],
                                identity=id_f8[:])
                            if m == 0:
                                nc.vector.tensor_copy(
                                    out=swT[:, m, 128 * i:128 * (i + 1)],
                                    in_=sp[:, :, 0])
                            else:
                                nc.scalar.activation(
                                    out=swT[:, m, 128 * i:128 * (i + 1)],
                                    in_=sp[:, :, 0], func=AF.Copy)
                    nc.sync.dma_start(out=ctx2[:],
                                      in_=ctx_d.ap().rearrange("h s d -> s h d"))
                    for h in range(H):
                        ctp = cxp.tile([DC, SC], f32, tag="ctp", name="ctp")
                        nc.tensor.transpose(out=ctp[:], in_=ctx2[:, h, :],
                                            identity=id_f[:SC, :SC])
                        nc.vector.tensor_copy(out=ctxT2[:, h, :], in_=ctp[:])
                    ckp = ckx.tile([D, H, SC], f32, tag="ckp")
                    for h in range(H):
                        nc.tensor.matmul(ckp[:, h, :], lhsT=W['wck'][:],
                                         rhs=ctxT2[:, h, :], start=True, stop=True)
                        cvwp = cxp.tile([SC, C], f32, tag="cvwp", name="cvwp")
                        nc.tensor.matmul(cvwp[:], lhsT=ctxT2[:, h, :],
                                         rhs=W['wcvw'][:, h, :],
                                         start=True, stop=not cv_bias)
                        if cv_bias:
                            nc.tensor.matmul(cvwp[:], lhsT=ones64[:1, :],
                                             rhs=W['bcvw'][:1, h, :],
                                             start=False, stop=True)
                        nc.vector.tensor_copy(out=cvw_sb[:, h, :], in_=cvwp[:])
                    nc.scalar.activation(out=ckT2[:], in_=ckp[:],
                                         func=AF.Identity, bias=W['bck'][:])

            if LVL >= 2:
                for h in range(H):
                    nc.sync.dma_start(out=st_head[:, h, :],
                                      in_=cc_out.ap()[32 * h:32 * (h + 1), :])

        if LVL == 2:
            shf = pers.tile([32, H, 257], f32, tag="shf")
            nc.vector.tensor_copy(out=shf[:], in_=st_head[:])
            for h in range(H):
                nc.sync.dma_start(out=out_d.ap()[32 * h:32 * (h + 1), 0, 0:256],
                                  in_=shf[:, h, 0:256])

        # ============ Phase D: slice-token attention (replicated) ============
        if LVL >= 3:
            with ExitStack() as ph:
                ds = ph.enter_context(tc.tile_pool(name="ds", bufs=2))
                rn = ds.tile([32, H], f32, tag="rn")
                nc.vector.tensor_scalar_add(out=rn[:], in0=st_head[:, :, 256],
                                            scalar1=EPS_SLICE)
                nc.vector.reciprocal(out=rn[:], in_=rn[:])
                for h in range(H):
                    nc.vector.tensor_scalar_mul(out=st_head[:, h, 0:256],
                                                in0=st_head[:, h, 0:256],
                                                scalar1=rn[:, h:h + 1])
                with tc.tile_pool(name="dpA", bufs=1, space="PSUM") as dpA:
                    stT_ps = dpA.tile([32, 256], bf16, tag="stT")
                    for h in range(H):
                        nc.tensor.transpose(
                            out=stT_ps[:, 32 * h:32 * (h + 1)],
                            in_=st_head[:, h, 32 * h:32 * (h + 1)],
                            identity=id_bf[:32, :32])
                    stT2 = ds.tile([32, 256], f32, tag="stT2")
                    nc.vector.tensor_copy(out=stT2[:], in_=stT_ps[:])
                    qkc = dpA.tile([32, 3, 256], f32, tag="qkc")
                    nc.tensor.matmul(qkc[:, 0, :], lhsT=W['wq'][:], rhs=stT2[:],
                                     start=True, stop=True)
                    nc.tensor.matmul(qkc[:, 1, :], lhsT=W['wk'][:], rhs=stT2[:],
                                     start=True, stop=True)
                    nc.tensor.matmul(qkc[:, 2, :], lhsT=W['wcq'][:], rhs=stT2[:],
                                     start=True, stop=True)
                    vw_ps = dpA.tile([32, H, 256], f32, tag="vw_ps")
                    for h in range(H):
                        nc.tensor.matmul(vw_ps[:, h, :],
                                         lhsT=stT2[:, 32 * h:32 * (h + 1)],
                                         rhs=W['wvw'][:, h, :],
                                         start=True, stop=True)
                    qT2 = ds.tile([32, 256], f32, tag="qT2")
                    nc.scalar.activation(out=qT2[:], in_=qkc[:, 0, :],
                                         func=AF.Copy, scale=float(D) ** -0.5)
                    kT2 = ds.tile([32, 256], f32, tag="kT2")
                    nc.vector.tensor_copy(out=kT2[:], in_=qkc[:, 1, :])
                    cqT2 = ds.tile([32, 256], f32, tag="cqT2")
                    nc.scalar.activation(out=cqT2[:], in_=qkc[:, 2, :],
                                         func=AF.Identity, bias=W['bcq'][:],
                                         scale=float(D) ** -0.5)
                    vw2 = ds.tile([32, H, 256], f32, tag="vw2")
                    nc.vector.tensor_copy(out=vw2[:], in_=vw_ps[:])
                dp = ph.enter_context(tc.tile_pool(name="dpB", bufs=1,
                                                   space="PSUM"))

                def softmax_rows(logits_ps, width, nheads, tag):
                    uu = ds.tile([32, nheads * width], f32, tag=tag + "u")
                    nc.scalar.activation(out=uu[:], in_=logits_ps[:], func=AF.Exp)
                    ss = ds.tile([32, nheads], f32, tag=tag + "s")
                    nc.vector.reduce_sum(out=ss[:], in_=uu[:].rearrange(
                        "p (h w) -> p h w", h=nheads), axis=AX.X)
                    nc.vector.reciprocal(out=ss[:], in_=ss[:])
                    nc.vector.tensor_tensor(
                        out=uu[:].rearrange("p (h w) -> p h w", h=nheads),
                        in0=uu[:].rearrange("p (h w) -> p h w", h=nheads),
                        in1=ss[:].broadcast_to([32, nheads, width]), op=ALU.mult)
                    return uu

                slp = dp.tile([32, 256], f32, tag="p32")
                for h in range(H):
                    sl = slice(32 * h, 32 * (h + 1))
                    nc.tensor.matmul(slp[:, sl], lhsT=qT2[:, sl], rhs=kT2[:, sl],
                                     start=True, stop=True)
                sattn = softmax_rows(slp, G, H, "sa")
                saT_ps = dp.tile([32, 256], f32, tag="p32", name="saT_ps")
                for h in range(H):
                    sl = slice(32 * h, 32 * (h + 1))
                    nc.tensor.transpose(out=saT_ps[:, sl], in_=sattn[:, sl],
                                        identity=id_f[:32, :32])
                saT = ds.tile([32, 256], f32, tag="saT")
                nc.vector.tensor_copy(out=saT[:], in_=saT_ps[:])

                clp = dp.tile([32, 512], f32, tag="p64")
                for h in range(H):
                    nc.tensor.matmul(clp[:, 64 * h:64 * (h + 1)],
                                     lhsT=cqT2[:, 32 * h:32 * (h + 1)],
                                     rhs=ckT2[:, h, :], start=True, stop=True)
                cattn = softmax_rows(clp, SC, H, "ca")
                caT_ps = dp.tile([64, 256], f32, tag="p64", name="caT_ps")
                for h in range(H):
                    nc.tensor.transpose(out=caT_ps[:, 32 * h:32 * (h + 1)],
                                        in_=cattn[:, 64 * h:64 * (h + 1)],
                                        identity=id_f[:32, :32])
                caT = ds.tile([64, 256], f32, tag="caT")
                nc.vector.tensor_copy(out=caT[:], in_=caT_ps[:])

                self_ps = dp.tile([128, 2, 256], f32, tag="self_ps")
                cross_ps = dp.tile([128, 2, 256], f32, tag="cross_ps")
                for h in range(H):
                    sl = slice(32 * h, 32 * (h + 1))
                    r0 = 32 * (h % 4)
                    nc.tensor.matmul(self_ps[r0:r0 + 32, h // 4, :],
                                     lhsT=saT[:, sl], rhs=vw2[:, h, :],
                                     start=True, stop=True,
                                     tile_position=(0, r0))
                    nc.tensor.matmul(cross_ps[r0:r0 + 32, h // 4, :],
                                     lhsT=caT[:, sl], rhs=cvw_sb[:, h, :],
                                     start=True, stop=True,
                                     tile_position=(0, r0))
                gtmp = ds.tile([128, 2, 256], f32, tag="gtmp")
                nc.vector.tensor_scalar_mul(out=gtmp[:], in0=self_ps[:],
                                            scalar1=W['mw'][:])
                if wout_bias:
                    nc.vector.tensor_add(out=gtmp[:], in0=gtmp[:],
                                         in1=W['bout8'][:])
                nc.vector.scalar_tensor_tensor(out=OW[:], in0=cross_ps[:],
                                               scalar=W['omw'][:], in1=gtmp[:],
                                               op0=ALU.mult, op1=ALU.add)

        if LVL == 3:
            owf = pers.tile([128, 2, 256], f32, tag="owf")
            nc.vector.tensor_copy(out=owf[:], in_=OW[:])
            for m in range(2):
                nc.sync.dma_start(out=out_d.ap()[:, m, :], in_=owf[:, m, :])

        # ============ Phase E: de-slice+Wout, LN2, MLP (merged pipeline) =====
        if LVL >= 4:
            with ExitStack() as ph:
                eio = ph.enter_context(tc.tile_pool(name="eio", bufs=3))
                esb = ph.enter_context(tc.tile_pool(name="esb", bufs=4))
                perse = ph.enter_context(tc.tile_pool(name="perse", bufs=1))
                o2p = ph.enter_context(tc.tile_pool(name="o2p", bufs=2,
                                                    space="PSUM"))
                ztp2 = ph.enter_context(tc.tile_pool(name="ztp2", bufs=2,
                                                     space="PSUM"))
                m1p = ph.enter_context(tc.tile_pool(name="m1p", bufs=2,
                                                    space="PSUM"))
                smp = ph.enter_context(tc.tile_pool(name="smp", bufs=2,
                                                    space="PSUM"))
                mv2 = perse.tile([128, NTILES, 2], f32, tag="mv2")
                rstd2 = perse.tile([128, NTILES], f32, tag="rstd2")
                wm2_4d = W['wm2'][:].rearrange("p (kp a c) -> p kp a c",
                                               kp=4, a=2)

                for scc in range(SCN):
                    tlo, thi = SCT * scc, SCT * (scc + 1)
                    for i in range(tlo, thi):
                        o2 = o2p.tile([128, C], f32, tag="o2", name="o2")
                        nc.tensor.matmul(
                            o2[:], lhsT=swT[:, :, 128 * i:128 * (i + 1)],
                            rhs=OW[:], start=True, stop=True, perf_mode=DR)
                        nc.vector.scalar_tensor_tensor(
                            out=fx_full(i), in0=o2[:], scalar=ROI,
                            in1=fx_full(i), op0=ALU.mult, op1=ALU.add)
                        st6 = esb.tile([128, 6], f32, tag="st6")
                        nc.vector.bn_stats(out=st6[:], in_=fx_full(i))
                        nc.vector.bn_aggr(out=mv2[:, i, :], in_=st6[:])
                    rsqrt_stats(esb, mv2[:, tlo:thi, :], thi - tlo,
                                [(rstd2[:, tlo:thi], 1.0)])
                    for ci in range(4 * scc, 4 * (scc + 1)):
                        z2T = eio.tile([128, 2, 512], fp8, tag="z2T")
                        for j in range(4):
                            i = 4 * ci + j
                            z2 = esb.tile([128, C], bf16, tag="z2")
                            nc.gpsimd.tensor_tensor(
                                out=z2[:], in0=fx_full(i),
                                in1=rstd2[:, i:i + 1].broadcast_to([128, C]),
                                op=ALU.mult)
                            zt_ps = ztp2.tile([128, 256], bf16, tag="z2t")
                            for k in range(2):
                                nc.tensor.transpose(
                                    out=zt_ps[:, 128 * k:128 * (k + 1)],
                                    in_=z2[:, 128 * k:128 * (k + 1)],
                                    identity=id_bf[:])
                            nc.vector.tensor_copy(
                                out=z2T[:, :, 128 * j:128 * (j + 1)],
                                in_=zt_ps[:].rearrange("p (k t) -> p k t", k=2))
                        m1T = eio.tile([128, 8, 512], fp8, tag="m1T")
                        for mt in range(8):
                            mp = m1p.tile([128, 512], f32, tag="m1", name="m1")
                            nc.tensor.matmul(
                                mp[:],
                                lhsT=W['wm1'][:, :, 128 * mt:128 * (mt + 1)],
                                rhs=z2T[:], start=True, stop=True,
                                perf_mode=DR)
                            if not m1_bias:
                                nc.scalar.activation(
                                    out=m1T[:, mt, :], in_=mp[:],
                                    func=(AF.Identity if sim else AF.Gelu),
                                    scale=1.0 / SM1)
                            else:
                                nc.scalar.activation(
                                    out=m1T[:, mt, :], in_=mp[:],
                                    func=(AF.Identity if sim else AF.Gelu),
                                    scale=1.0 / SM1, bias=W['bm1c'][mt][:])
                        o_t = eio.tile([128, 4, C], f32, tag="ot")
                        for j in range(4):
                            i = 4 * ci + j
                            m2ps = smp.tile([128, C], f32, tag="m2", name="m2ps")
                            for kp in range(4):
                                nc.tensor.matmul(
                                    m2ps[:],
                                    lhsT=m1T[:, 2 * kp:2 * kp + 2,
                                             128 * j:128 * (j + 1)],
                                    rhs=wm2_4d[:, kp, :, :],
                                    start=(kp == 0), stop=(kp == 3),
                                    perf_mode=DR)
                            nc.vector.scalar_tensor_tensor(
                                out=o_t[:, j, :], in0=m2ps[:], scalar=1.0 / SM2,
                                in1=fx_full(i), op0=ALU.mult, op1=ALU.add)
                            if m2_bias:
                                nc.vector.tensor_add(out=o_t[:, j, :],
                                                     in0=o_t[:, j, :],
                                                     in1=W['bm2'][:])
                        nc.sync.dma_start(
                            out=out_d.ap()[:, 4 * ci:4 * (ci + 1), :],
                            in_=o_t[:])

    nc.compile()
    return nc


def _prep_inputs(NT, inputs):
    """Host-side weight folding + per-core input maps."""
    f = lambda x: np.asarray(x, np.float32)
    g1 = f(inputs["ln1_g"]); b1 = f(inputs["ln1_b"])
    g2 = f(inputs["ln2_g"]); b2 = f(inputs["ln2_b"])
    Wfx = f(inputs["Wfx"]); bfx = f(inputs["bfx"])
    Wx = f(inputs["Wx"]); bx = f(inputs["bx"])
    Wslice = f(inputs["Wslice"]); bslice = f(inputs["bslice"])
    temp = f(inputs["temperature"]).reshape(H)
    Wm1 = f(inputs["Wm1"]); bm1 = f(inputs["bm1"])
    Wm2 = f(inputs["Wm2"]); bm2 = f(inputs["bm2"])
    Wout = f(inputs["Wout"]); bout = f(inputs["bout"])
    Wq = f(inputs["Wq"]); Wk = f(inputs["Wk"]); Wv = f(inputs["Wv"])
    Wcq = f(inputs["Wcq"]); bcq = f(inputs["bcq"])
    Wck = f(inputs["Wck"]); bck = f(inputs["bck"])
    Wcv = f(inputs["Wcv"]); bcv = f(inputs["bcv"])
    scale = float(D) ** -0.5

    # block-diag Wslice scaled by 1/temperature
    Wbd = np.zeros((H * D, H * G), np.float32)
    for h in range(H):
        Wbd[h * D:(h + 1) * D, h * G:(h + 1) * G] = Wslice / temp[h]
    bslice_rep = np.concatenate([bslice / temp[h] for h in range(H)])

    # column-centered, fp8-scaled fused projection weights:
    # (x - mean(x)) @ W == x @ (W - colmean(W))
    Wfxp = g1[:, None] * Wfx
    Wxs = (g1[:, None] * Wx) @ Wbd
    Wfxp_c = (Wfxp - Wfxp.mean(0, keepdims=True)) * SA
    Wxs_c = (Wxs - Wxs.mean(0, keepdims=True)) * SB
    wall_full = np.concatenate([Wfxp_c, Wxs_c], axis=1)       # [C, 512]
    wall_dr = np.ascontiguousarray(
        wall_full.reshape(2, 128, 512).transpose(1, 0, 2)).astype(F8)
    ball_f = np.concatenate([(b1 @ Wfx + bfx) * SA,
                             ((b1 @ Wx + bx) @ Wbd + bslice_rep) * SB])
    ball = ball_f[None, :].astype(BF)
    proj_bias = bool(np.any(ball_f != 0.0))

    # MLP weights: column-centered g2*Wm1, fp8 DR layouts
    W1 = g2[:, None] * Wm1
    W1_c = (W1 - W1.mean(0, keepdims=True)) * SM1              # [C, 4C]
    wm1_dr = np.ascontiguousarray(
        W1_c.reshape(2, 128, 4 * C).transpose(1, 0, 2)).astype(F8)
    bm1p = (b2 @ Wm1 + bm1).astype(np.float32)
    m1_bias = bool(np.any(bm1p != 0.0))
    wm2_dr = np.ascontiguousarray(
        (Wm2 * SM2).reshape(4, 2, 128, C).transpose(2, 0, 1, 3)
        .reshape(128, 8 * C)).astype(F8)
    bm2_rep = np.ascontiguousarray(
        np.broadcast_to(bm2[None, :], (128, C)), np.float32)
    m2_bias = bool(np.any(bm2 != 0.0))

    # Wout folded into attention values (scaled by SOW for fp8 range)
    Wvw = np.stack([Wv @ Wout[32 * h:32 * (h + 1), :] for h in range(H)],
                   1) * SOW
    Wcvw = np.stack([Wcv @ Wout[32 * h:32 * (h + 1), :] for h in range(H)],
                    1) * SOW
    bcvw = np.stack([bcv @ Wout[32 * h:32 * (h + 1), :] for h in range(H)],
                    0) * SOW
    cv_bias = bool(np.any(bcv != 0.0))
    bout8 = np.ascontiguousarray(
        np.broadcast_to(bout[None, None, :] * (SOW / H), (128, 2, C)),
        np.float32)
    wout_bias = bool(np.any(bout != 0.0))

    mwv = float(1.0 / (1.0 + np.exp(-f(inputs["state_mixing"]))))
    mw = np.full((128, 1), mwv, np.float32)
    omw = np.full((128, 1), 1.0 - mwv, np.float32)

    id_f = np.eye(128, dtype=np.float32)

    common = dict(
        wall=wall_dr, ball=ball, wm1=wm1_dr, bm1=bm1p, wm2=wm2_dr,
        bm2=bm2_rep, bout8=bout8,
        wq=Wq, wk=Wk, wcq=Wcq, bcq=(bcq * scale).astype(np.float32),
        wck=Wck, bck=bck,
        wvw=np.ascontiguousarray(Wvw, np.float32),
        wcvw=np.ascontiguousarray(Wcvw, np.float32),
        bcvw=np.ascontiguousarray(bcvw[None, :, :], np.float32),
        mw=mw, omw=omw,
        id_bf=id_f.astype(BF), id_f8=id_f.astype(F8), id_f=id_f,
        ones_bf=np.ones((1, 128), BF), ones64=np.ones((1, 64), np.float32),
    )

    fx = f(inputs["fx"])
    ctxt = f(inputs["context"])
    in_maps = []
    for core in range(NCORES):
        b, s = core // CPB, core % CPB
        x = fx[b, s * NT:(s + 1) * NT, :]                      # [NT, C]
        m = dict(common)
        m["fxq"] = np.ascontiguousarray(
            x.reshape(NT // 128, 128, C).transpose(1, 0, 2))
        m["fxT"] = np.ascontiguousarray(
            x.T.reshape(2, 128, NT).transpose(1, 0, 2)).astype(F8)
        m["ctx"] = np.ascontiguousarray(ctxt[b])
        in_maps.append(m)
    return in_maps, (proj_bias, m1_bias, wout_bias, m2_bias, cv_bias)


_CACHE = {}


def _get_compiled(NT, flags):
    key = (NT,) + flags
    if key not in _CACHE:
        _CACHE[key] = _build(NT, flags)
    return _CACHE[key]


def kernel(**inputs):
    from concourse.bass_utils import run_bass_kernel_spmd
    NT = NT_FULL
    in_maps, flags = _prep_inputs(NT, inputs)
    nc = _get_compiled(NT, flags)
    res = run_bass_kernel_spmd(nc, in_maps, list(range(NCORES)))
    out = np.empty((B, N, C), np.float32)
    for core in range(NCORES):
        b, s = core // CPB, core % CPB
        out[b, s * NT:(s + 1) * NT, :] = (
            res.results[core]["out"].transpose(1, 0, 2).reshape(NT, C))
    return out


# revision 25
# speedup vs baseline: 1.2242x; 1.0151x over previous
"""Bass/Trainium2 kernel for nn_GALE_block (dense_transformer, 8 NeuronCores).

Sharding: data-parallel over B (2 groups of 4 cores), sequence-parallel over N
within each group (8192 tokens/core).  Slice-token statistics are combined with
one small AllReduce per group; the tiny slice attention is replicated; the
de-slice + output projection + MLP are fully local.

Structural choices vs a straightforward mapping:
  - LN mean-subtraction folds into column-centered weights host-side
    ((x-m)@W == x@(W-colmean W)); the rstd scale applies post-matmul.  The
    host supplies x token-major (f32, stats/residual) AND feature-major
    (fp8), eliminating all phase-B PE transposes.
  - Wout folds into the slice-attention values host-side (Wvw = Wv@Wout_h,
    Wcvw = Wcv@Wout_h), so de-slice + output projection collapse into one
    256-contraction matmul per tile against OW[hg,c] = mix of attn outputs.
  - All large matmuls run fp8 DoubleRow (2x PE throughput).  Host scales
    weights into fp8 range (SA/SB/SM1/SM2/SOW/SSW); inverse scales fold into
    activation scale factors and the residual-add constants.
  - Elementwise work is spread across Scalar/Vector/GpSimd.
"""

import numpy as np
import ml_dtypes

# problem dims (hardcoded per contest contract)
B, N, C, H, D, G, SC, DC = 2, 32768, 256, 8, 32, 32, 64, 32
NCORES = 8
CPB = 4                      # cores per batch entry
NT_FULL = N // CPB           # tokens per core = 8192
RG = [[0, 1, 2, 3], [4, 5, 6, 7]]
EPS_LN = 1e-5
EPS_SLICE = 1e-5

BF = ml_dtypes.bfloat16
F8 = ml_dtypes.float8_e4m3

SA = 16.0    # Wfx branch fp8 weight scale
SB = 64.0    # Wslice branch fp8 weight scale
SM1 = 16.0   # Wm1 fp8 weight scale
SM2 = 16.0   # Wm2 fp8 weight scale
SOW = 128.0  # attention-value (Wvw/Wcvw) fp8 scale
SSW = 1.0    # slice-weight (sw) fp8 scale (1 = rely on fp8 subnormals)


def _build(NT, flags, sim=False, cut=None):
    """Build the SPMD Bass program for NT tokens/core."""
    proj_bias, m1_bias, wout_bias, m2_bias, cv_bias = flags
    import concourse.bass as bass
    import concourse.bacc as bacc
    import concourse.mybir as mybir
    import concourse.tile as tile
    from contextlib import ExitStack

    f32 = mybir.dt.float32
    bf16 = mybir.dt.bfloat16
    fp8 = mybir.dt.float8e4
    AF = mybir.ActivationFunctionType
    ALU = mybir.AluOpType
    AX = mybir.AxisListType
    DR = mybir.MatmulPerfMode.DoubleRow

    LVL = {"B": 1, "C": 2, "D": 3}.get(cut, 4)
    NTILES = NT // 128
    SCN = 4                       # super-chunks for phase E
    SCT = NTILES // SCN           # tiles per super-chunk

    nc = bacc.Bacc("TRN2", target_bir_lowering=False, debug=False,
                   num_devices=NCORES)

    def din(name, shape, dt=f32):
        return nc.dram_tensor(name, shape, dt, kind="ExternalInput")

    # ---- inputs (host pre-folds weights; see _prep_inputs) ----
    fxq_d = din("fxq", [128, NTILES, C])          # token-major pretiled f32
    fxT_d = din("fxT", [128, 2, NT], fp8)         # feature-major fp8
    ctx_d = din("ctx", [H, SC, DC])
    wall_d = din("wall", [128, 2, 512], fp8)      # centered [Wfx'|Wxs] DR fp8
    ball_d = din("ball", [1, 512], bf16)          # fused proj bias row (rare)
    wm1_d = din("wm1", [128, 2, 4 * C], fp8)      # centered g2Wm1*SM1 DR fp8
    bm1_d = din("bm1", [4 * C])                   # fused col bias (fp32)
    wm2_d = din("wm2", [128, 8 * C], fp8)         # Wm2*SM2 DR fp8 [p,(kp a c)]
    bm2_d = din("bm2", [128, C])                  # bm2 replicated (rare)
    bout8_d = din("bout8", [128, 2, C])           # bout*SOW/H replicated (rare)
    wq_d = din("wq", [D, D])
    wk_d = din("wk", [D, D])
    wcq_d = din("wcq", [D, D])
    bcq_d = din("bcq", [D])                       # pre-scaled by D^-0.5
    wck_d = din("wck", [DC, D])
    bck_d = din("bck", [D])
    wvw_d = din("wvw", [D, H, C])                 # Wv @ Wout_h * SOW
    wcvw_d = din("wcvw", [DC, H, C])              # Wcv @ Wout_h * SOW
    bcvw_d = din("bcvw", [1, H, C])               # bcv @ Wout_h * SOW (rare)
    mw_d = din("mw", [128, 1])                    # sigmoid(state_mixing) bcast
    omw_d = din("omw", [128, 1])                  # 1 - mw
    id_bf_d = din("id_bf", [128, 128], bf16)
    id_f8_d = din("id_f8", [128, 128], fp8)
    id_f_d = din("id_f", [128, 128])
    ones_bf_d = din("ones_bf", [1, 128], bf16)
    ones64_d = din("ones64", [1, 64])

    out_d = nc.dram_tensor("out", [128, NTILES, C], f32, kind="ExternalOutput")

    cc_in = nc.dram_tensor("cc_in", [2 * 128, 257], bf16)
    cc_out = nc.dram_tensor("cc_out", [2 * 128, 257], bf16)
    ccd_in = nc.dram_tensor("ccd_in", [1, 1], bf16)
    ccd_out = nc.dram_tensor("ccd_out", [1, 1], bf16)

    ROI = 1.0 / (SOW * SSW)      # o2 unscale const

    with tile.TileContext(nc) as tc, ExitStack() as big:
        wp = big.enter_context(tc.tile_pool(name="wp", bufs=1))
        pers = big.enter_context(tc.tile_pool(name="pers", bufs=1))

        def load(pool, shape, dt, src_ap, tag, eng=None):
            t = pool.tile(shape, dt, tag=tag, name=tag)
            (eng or nc.sync).dma_start(out=t[:], in_=src_ap)
            return t

        wall = load(wp, [128, 2, 512], fp8, wall_d.ap(), "wall")
        ball = load(wp, [1, 512], bf16, ball_d.ap(), "ball")
        id_bf = load(wp, [128, 128], bf16, id_bf_d.ap(), "id_bf")
        id_f8 = load(wp, [128, 128], fp8, id_f8_d.ap(), "id_f8")
        id_f = load(wp, [128, 128], f32, id_f_d.ap(), "id_f")
        ones_bf = load(wp, [1, 128], bf16, ones_bf_d.ap(), "ones_bf")
        ones64 = load(wp, [1, 64], f32, ones64_d.ap(), "ones64")

        def rsqrt_stats(pool, mv_sl, w, outs):
            # outs: list of (dest_slice, mult_const); rstd = rsqrt(var+eps)
            # by Newton from a constant seed (valid for var+eps < 3).
            xe = pool.tile([128, w], f32, tag="rsq_xe", name="rsq_xe")
            y = pool.tile([128, w], f32, tag="rsq_y", name="rsq_y")
            t = pool.tile([128, w], f32, tag="rsq_t", name="rsq_t")
            nc.vector.tensor_scalar_add(out=xe[:], in0=mv_sl[:, :, 1],
                                        scalar1=EPS_LN)
            nc.vector.memset(y[:], 1.0)
            for _ in range(5):  # Newton: y *= 1.5 - 0.5*x*y*y
                nc.vector.tensor_mul(out=t[:], in0=y[:], in1=y[:])
                nc.vector.tensor_mul(out=t[:], in0=t[:], in1=xe[:])
                nc.vector.tensor_scalar_mul(out=t[:], in0=t[:], scalar1=-0.5)
                nc.vector.tensor_scalar_add(out=t[:], in0=t[:], scalar1=1.5)
                nc.vector.tensor_mul(out=y[:], in0=y[:], in1=t[:])
            for dest, mult in outs:
                if mult == 1.0:
                    nc.vector.tensor_copy(out=dest, in_=y[:])
                else:
                    nc.vector.tensor_scalar_mul(out=dest, in0=y[:],
                                                scalar1=mult)

        W = {}

        def late_loads():
            W['wm1'] = load(wp, [128, 2, 4 * C], fp8, wm1_d.ap(), "wm1",
                            nc.scalar)
            W['bm1c'] = [load(wp, [128, 1], f32,
                              bass.AP(bm1_d, 128 * m, [[1, 128], [1, 1]]),
                              f"bm1c{m}", nc.scalar) for m in range(8)]
            W['wm2'] = load(wp, [128, 8 * C], fp8, wm2_d.ap(), "wm2", nc.scalar)
            if m2_bias:
                W['bm2'] = load(wp, [128, C], f32, bm2_d.ap(), "bm2", nc.scalar)
            if wout_bias:
                W['bout8'] = load(wp, [128, 2, C], f32, bout8_d.ap(), "bout8",
                                  nc.scalar)
            W['wq'] = load(wp, [D, D], f32, wq_d.ap(), "wq", nc.scalar)
            W['wk'] = load(wp, [D, D], f32, wk_d.ap(), "wk", nc.scalar)
            W['wcq'] = load(wp, [D, D], f32, wcq_d.ap(), "wcq", nc.scalar)
            W['bcq'] = load(wp, [D, 1], f32,
                            bass.AP(bcq_d, 0, [[1, D], [1, 1]]), "bcq", nc.scalar)
            W['wck'] = load(wp, [DC, D], f32, wck_d.ap(), "wck", nc.scalar)
            W['bck'] = load(wp, [D, 1], f32,
                            bass.AP(bck_d, 0, [[1, D], [1, 1]]), "bck", nc.scalar)
            W['wvw'] = load(wp, [D, H, C], f32, wvw_d.ap(), "wvw", nc.scalar)
            W['wcvw'] = load(wp, [DC, H, C], f32, wcvw_d.ap(), "wcvw", nc.scalar)
            if cv_bias:
                W['bcvw'] = load(wp, [1, H, C], f32, bcvw_d.ap(), "bcvw",
                                 nc.scalar)
            W['mw'] = load(wp, [128, 1], f32, mw_d.ap(), "mw", nc.scalar)
            W['omw'] = load(wp, [128, 1], f32, omw_d.ap(), "omw", nc.scalar)

        # persistent activation buffers
        NQ = max(NTILES // 16, 1)
        fx_q = [pers.tile([128, min(16, NTILES), C], f32, tag=f"fx_q{q}",
                          name=f"fx_q{q}") for q in range(NQ)]

        def fx_full(i):
            return fx_q[i // 16][:, i % 16, :]

        fxT = pers.tile([128, 2, NT], fp8, tag="fxT")
        swtok = pers.tile([128, NTILES, 256], fp8, tag="swtok")
        swT = pers.tile([128, 2, NT], fp8, tag="swT")
        stp = pers.tile([128, 2, 257], bf16, tag="stp")
        st_head = pers.tile([32, H, 257], bf16, tag="st_head")
        OW = pers.tile([128, 2, C], fp8, tag="OW")
        ctx2 = pers.tile([64, H, DC], f32, tag="ctx2")
        ctxT2 = pers.tile([DC, H, SC], f32, tag="ctxT2")
        ckT2 = pers.tile([D, H, SC], f32, tag="ckT2")
        cvw_sb = pers.tile([SC, H, C], f32, tag="cvw_sb")

        # ============ Phase B: stats, projections, slice weights, st ==========
        with ExitStack() as ph:
            persb = ph.enter_context(tc.tile_pool(name="persb", bufs=1))
            io = ph.enter_context(tc.tile_pool(name="io", bufs=3))
            sb = ph.enter_context(tc.tile_pool(name="sb", bufs=4))
            phb = ExitStack()
            prj = phb.enter_context(tc.tile_pool(name="prj", bufs=4, space="PSUM"))
            stps = phb.enter_context(tc.tile_pool(name="stps", bufs=1, space="PSUM"))

            mvall = persb.tile([128, NTILES, 2], f32, tag="mvall")
            rstdA = persb.tile([128, NTILES], f32, tag="rstdA")
            rstdB = persb.tile([128, NTILES], f32, tag="rstdB")
            fxm4 = persb.tile([128, 4, 2, 257], fp8, tag="fxm4")
            nc.vector.memset(fxm4[:, :, :, 256:257], 1.0)
            st_ps = [stps.tile([128, 257], f32, tag=f"st{m}", name=f"st{m}")
                     for m in range(2)]

            BSZ = min(16, NTILES)

            def stats_batch(lo, hi):
                q = lo // 16
                nc.sync.dma_start(out=fxT[:, :, 128 * lo:128 * hi],
                                  in_=fxT_d.ap()[:, :, 128 * lo:128 * hi])
                nc.sync.dma_start(out=fx_q[q][:, 0:hi - lo, :],
                                  in_=fxq_d.ap()[:, lo:hi, :])
                for i in range(lo, hi):
                    st6 = sb.tile([128, 6], f32, tag="st6")
                    nc.vector.bn_stats(out=st6[:], in_=fx_full(i))
                    nc.vector.bn_aggr(out=mvall[:, i, :], in_=st6[:])
                rsqrt_stats(sb, mvall[:, lo:hi, :], hi - lo,
                            [(rstdA[:, lo:hi], 1.0 / SA),
                             (rstdB[:, lo:hi], 1.0 / SB)])

            def tiles_interleaved():
                for lo in range(0, NTILES, BSZ):
                    stats_batch(lo, min(lo + BSZ, NTILES))
                    if lo == 0:
                        late_loads()
                    yield from range(lo, min(lo + BSZ, NTILES))

            for i in tiles_interleaved():
                pj = prj.tile([128, 512], f32, tag="pj")
                nc.tensor.matmul(pj[:], lhsT=fxT[:, :, 128 * i:128 * (i + 1)],
                                 rhs=wall[:], start=True, stop=not proj_bias,
                                 perf_mode=DR)
                if proj_bias:
                    nc.tensor.matmul(pj[:], lhsT=ones_bf[:1, :], rhs=ball[:1, :],
                                     start=False, stop=True,
                                     skip_group_check=True)
                bsl = (i // 2) % 4
                nc.scalar.activation(out=fxm4[:, bsl, i % 2, 0:256],
                                     in_=pj[:, 0:256],
                                     func=AF.Copy, scale=rstdA[:, i:i + 1])
                u = io.tile([128, 256], f32, tag="u")
                nc.scalar.activation(out=u[:], in_=pj[:, 256:512], func=AF.Exp,
                                     scale=rstdB[:, i:i + 1])
                s8 = sb.tile([128, 8], f32, tag="s8")
                nc.vector.reduce_sum(out=s8[:], in_=u[:].rearrange(
                    "p (h g) -> p h g", h=H), axis=AX.X)
                nc.vector.reciprocal(out=s8[:], in_=s8[:])
                nc.gpsimd.tensor_tensor(
                    out=swtok[:, i, :].rearrange("p (h g) -> p h g", h=H),
                    in0=u[:].rearrange("p (h g) -> p h g", h=H),
                    in1=s8[:].broadcast_to([128, H, G]), op=ALU.mult)
                if i % 2 == 1:
                    for m in range(2):
                        nc.tensor.matmul(
                            st_ps[m][:],
                            lhsT=swtok[:, i - 1:i + 1, 128 * m:128 * (m + 1)],
                            rhs=fxm4[:, bsl, :, :],
                            start=(i == 1), stop=(i == NTILES - 1),
                            perf_mode=DR, skip_group_check=True)

            for m in range(2):
                nc.vector.tensor_copy(out=stp[:, m, :], in_=st_ps[m][:])
            phb.close()

            if LVL == 1:
                stpf = persb.tile([128, 2, 257], f32, tag="stpf")
                nc.vector.tensor_copy(out=stpf[:], in_=stp[:])
                for m in range(2):
                    nc.sync.dma_start(out=out_d.ap()[:, m, 0:256],
                                      in_=stpf[:, m, 0:256])

            # ============ Phase C: AllReduce of slice partials ============
            if LVL >= 2:
                for m in range(2):
                    nc.sync.dma_start(out=cc_in.ap()[128 * m:128 * (m + 1), :],
                                      in_=stp[:, m, :])
                nc.sync.dma_start(out=ccd_in.ap(), in_=stp[0:1, 0, 0:1])
                nc.gpsimd.collective_compute(
                    "AllReduce", ALU.add, ins=[cc_in.ap()], outs=[cc_out.ap()],
                    replica_groups=RG)
                # trailing dummy absorbs the ~20ms completion-poll quantum of
                # the LAST collective in this runtime
                nc.gpsimd.collective_compute(
                    "AllReduce", ALU.add, ins=[ccd_in.ap()], outs=[ccd_out.ap()],
                    replica_groups=RG)

            # --- overlap window: sw -> swT transposes + context prep ---
            if LVL >= 2:
                with ExitStack() as ph2:
                    trp = ph2.enter_context(
                        tc.tile_pool(name="trp", bufs=3, space="PSUM"))
                    cxp = ph2.enter_context(
                        tc.tile_pool(name="cxp", bufs=2, space="PSUM"))
                    ckx = ph2.enter_context(
                        tc.tile_pool(name="ckx", bufs=1, space="PSUM"))
                    for i2 in range(0, NTILES, 2):
                        for m in range(2):
                            sp = trp.tile([128, 256, 2], fp8, tag="swt",
                                          name="swt")
                            for jj in range(2):
                                nc.tensor.transpose(
                                    out=sp[:, 128 * jj:128 * (jj + 1), 0],
                                    in_=swtok[:, i2 + jj,
                                              128 * m:128 * (m + 1)],
                                    identity=id_f8[:])
                            if m == 0:
                                nc.vector.tensor_copy(
                                    out=swT[:, m, 128 * i2:128 * (i2 + 2)],
                                    in_=sp[:, :, 0])
                            else:
                                nc.scalar.activation(
                                    out=swT[:, m, 128 * i2:128 * (i2 + 2)],
                                    in_=sp[:, :, 0], func=AF.Copy)
                    nc.sync.dma_start(out=ctx2[:],
                                      in_=ctx_d.ap().rearrange("h s d -> s h d"))
                    for h in range(H):
                        ctp = cxp.tile([DC, SC], f32, tag="ctp", name="ctp")
                        nc.tensor.transpose(out=ctp[:], in_=ctx2[:, h, :],
                                            identity=id_f[:SC, :SC])
                        nc.vector.tensor_copy(out=ctxT2[:, h, :], in_=ctp[:])
                    ckp = ckx.tile([D, H, SC], f32, tag="ckp")
                    for h in range(H):
                        nc.tensor.matmul(ckp[:, h, :], lhsT=W['wck'][:],
                                         rhs=ctxT2[:, h, :], start=True, stop=True)
                        cvwp = cxp.tile([SC, C], f32, tag="cvwp", name="cvwp")
                        nc.tensor.matmul(cvwp[:], lhsT=ctxT2[:, h, :],
                                         rhs=W['wcvw'][:, h, :],
                                         start=True, stop=not cv_bias)
                        if cv_bias:
                            nc.tensor.matmul(cvwp[:], lhsT=ones64[:1, :],
                                             rhs=W['bcvw'][:1, h, :],
                                             start=False, stop=True)
                        nc.vector.tensor_copy(out=cvw_sb[:, h, :], in_=cvwp[:])
                    nc.scalar.activation(out=ckT2[:], in_=ckp[:],
                                         func=AF.Identity, bias=W['bck'][:])

            if LVL >= 2:
                for h in range(H):
                    nc.sync.dma_start(out=st_head[:, h, :],
                                      in_=cc_out.ap()[32 * h:32 * (h + 1), :])

        if LVL == 2:
            shf = pers.tile([32, H, 257], f32, tag="shf")
            nc.vector.tensor_copy(out=shf[:], in_=st_head[:])
            for h in range(H):
                nc.sync.dma_start(out=out_d.ap()[32 * h:32 * (h + 1), 0, 0:256],
                                  in_=shf[:, h, 0:256])

        # ============ Phase D: slice-token attention (replicated) ============
        if LVL >= 3:
            with ExitStack() as ph:
                ds = ph.enter_context(tc.tile_pool(name="ds", bufs=2))
                rn = ds.tile([32, H], f32, tag="rn")
                nc.vector.tensor_scalar_add(out=rn[:], in0=st_head[:, :, 256],
                                            scalar1=EPS_SLICE)
                nc.vector.reciprocal(out=rn[:], in_=rn[:])
                for h in range(H):
                    nc.vector.tensor_scalar_mul(out=st_head[:, h, 0:256],
                                                in0=st_head[:, h, 0:256],
                                                scalar1=rn[:, h:h + 1])
                with tc.tile_pool(name="dpA", bufs=1, space="PSUM") as dpA:
                    stT_ps = dpA.tile([32, 256], bf16, tag="stT")
                    for h in range(H):
                        nc.tensor.transpose(
                            out=stT_ps[:, 32 * h:32 * (h + 1)],
                            in_=st_head[:, h, 32 * h:32 * (h + 1)],
                            identity=id_bf[:32, :32])
                    stT2 = ds.tile([32, 256], f32, tag="stT2")
                    nc.vector.tensor_copy(out=stT2[:], in_=stT_ps[:])
                    qkc = dpA.tile([32, 3, 256], f32, tag="qkc")
                    nc.tensor.matmul(qkc[:, 0, :], lhsT=W['wq'][:], rhs=stT2[:],
                                     start=True, stop=True)
                    nc.tensor.matmul(qkc[:, 1, :], lhsT=W['wk'][:], rhs=stT2[:],
                                     start=True, stop=True)
                    nc.tensor.matmul(qkc[:, 2, :], lhsT=W['wcq'][:], rhs=stT2[:],
                                     start=True, stop=True)
                    vw_ps = dpA.tile([32, H, 256], f32, tag="vw_ps")
                    for h in range(H):
                        nc.tensor.matmul(vw_ps[:, h, :],
                                         lhsT=stT2[:, 32 * h:32 * (h + 1)],
                                         rhs=W['wvw'][:, h, :],
                                         start=True, stop=True)
                    qT2 = ds.tile([32, 256], f32, tag="qT2")
                    nc.scalar.activation(out=qT2[:], in_=qkc[:, 0, :],
                                         func=AF.Copy, scale=float(D) ** -0.5)
                    kT2 = ds.tile([32, 256], f32, tag="kT2")
                    nc.vector.tensor_copy(out=kT2[:], in_=qkc[:, 1, :])
                    cqT2 = ds.tile([32, 256], f32, tag="cqT2")
                    nc.scalar.activation(out=cqT2[:], in_=qkc[:, 2, :],
                                         func=AF.Identity, bias=W['bcq'][:],
                                         scale=float(D) ** -0.5)
                    vw2 = ds.tile([32, H, 256], f32, tag="vw2")
                    nc.vector.tensor_copy(out=vw2[:], in_=vw_ps[:])
                dp = ph.enter_context(tc.tile_pool(name="dpB", bufs=1,
                                                   space="PSUM"))

                def softmax_rows(logits_ps, width, nheads, tag):
                    uu = ds.tile([32, nheads * width], f32, tag=tag + "u")
                    nc.scalar.activation(out=uu[:], in_=logits_ps[:], func=AF.Exp)
                    ss = ds.tile([32, nheads], f32, tag=tag + "s")
                    nc.vector.reduce_sum(out=ss[:], in_=uu[:].rearrange(
                        "p (h w) -> p h w", h=nheads), axis=AX.X)
                    nc.vector.reciprocal(out=ss[:], in_=ss[:])
                    nc.vector.tensor_tensor(
                        out=uu[:].rearrange("p (h w) -> p h w", h=nheads),
                        in0=uu[:].rearrange("p (h w) -> p h w", h=nheads),
                        in1=ss[:].broadcast_to([32, nheads, width]), op=ALU.mult)
                    return uu

                slp = dp.tile([32, 256], f32, tag="p32")
                for h in range(H):
                    sl = slice(32 * h, 32 * (h + 1))
                    nc.tensor.matmul(slp[:, sl], lhsT=qT2[:, sl], rhs=kT2[:, sl],
                                     start=True, stop=True)
                sattn = softmax_rows(slp, G, H, "sa")
                saT_ps = dp.tile([32, 256], f32, tag="p32", name="saT_ps")
                for h in range(H):
                    sl = slice(32 * h, 32 * (h + 1))
                    nc.tensor.transpose(out=saT_ps[:, sl], in_=sattn[:, sl],
                                        identity=id_f[:32, :32])
                saT = ds.tile([32, 256], f32, tag="saT")
                nc.vector.tensor_copy(out=saT[:], in_=saT_ps[:])

                clp = dp.tile([32, 512], f32, tag="p64")
                for h in range(H):
                    nc.tensor.matmul(clp[:, 64 * h:64 * (h + 1)],
                                     lhsT=cqT2[:, 32 * h:32 * (h + 1)],
                                     rhs=ckT2[:, h, :], start=True, stop=True)
                cattn = softmax_rows(clp, SC, H, "ca")
                caT_ps = dp.tile([64, 256], f32, tag="p64", name="caT_ps")
                for h in range(H):
                    nc.tensor.transpose(out=caT_ps[:, 32 * h:32 * (h + 1)],
                                        in_=cattn[:, 64 * h:64 * (h + 1)],
                                        identity=id_f[:32, :32])
                caT = ds.tile([64, 256], f32, tag="caT")
                nc.vector.tensor_copy(out=caT[:], in_=caT_ps[:])

                self_ps = dp.tile([128, 2, 256], f32, tag="self_ps")
                cross_ps = dp.tile([128, 2, 256], f32, tag="cross_ps")
                for h in range(H):
                    sl = slice(32 * h, 32 * (h + 1))
                    r0 = 32 * (h % 4)
                    nc.tensor.matmul(self_ps[r0:r0 + 32, h // 4, :],
                                     lhsT=saT[:, sl], rhs=vw2[:, h, :],
                                     start=True, stop=True,
                                     tile_position=(0, r0))
                    nc.tensor.matmul(cross_ps[r0:r0 + 32, h // 4, :],
                                     lhsT=caT[:, sl], rhs=cvw_sb[:, h, :],
                                     start=True, stop=True,
                                     tile_position=(0, r0))
                gtmp = ds.tile([128, 2, 256], f32, tag="gtmp")
                nc.vector.tensor_scalar_mul(out=gtmp[:], in0=self_ps[:],
                                            scalar1=W['mw'][:])
                if wout_bias:
                    nc.vector.tensor_add(out=gtmp[:], in0=gtmp[:],
                                         in1=W['bout8'][:])
                nc.vector.scalar_tensor_tensor(out=OW[:], in0=cross_ps[:],
                                               scalar=W['omw'][:], in1=gtmp[:],
                                               op0=ALU.mult, op1=ALU.add)

        if LVL == 3:
            owf = pers.tile([128, 2, 256], f32, tag="owf")
            nc.vector.tensor_copy(out=owf[:], in_=OW[:])
            for m in range(2):
                nc.sync.dma_start(out=out_d.ap()[:, m, :], in_=owf[:, m, :])

        # ============ Phase E: de-slice+Wout, LN2, MLP (merged pipeline) =====
        if LVL >= 4:
            with ExitStack() as ph:
                eio = ph.enter_context(tc.tile_pool(name="eio", bufs=3))
                esb = ph.enter_context(tc.tile_pool(name="esb", bufs=4))
                perse = ph.enter_context(tc.tile_pool(name="perse", bufs=1))
                o2p = ph.enter_context(tc.tile_pool(name="o2p", bufs=2,
                                                    space="PSUM"))
                ztp2 = ph.enter_context(tc.tile_pool(name="ztp2", bufs=2,
                                                     space="PSUM"))
                m1p = ph.enter_context(tc.tile_pool(name="m1p", bufs=2,
                                                    space="PSUM"))
                smp = ph.enter_context(tc.tile_pool(name="smp", bufs=2,
                                                    space="PSUM"))
                mv2 = perse.tile([128, NTILES, 2], f32, tag="mv2")
                rstd2 = perse.tile([128, NTILES], f32, tag="rstd2")
                wm2_4d = W['wm2'][:].rearrange("p (kp a c) -> p kp a c",
                                               kp=4, a=2)

                for scc in range(SCN):
                    tlo, thi = SCT * scc, SCT * (scc + 1)
                    for i in range(tlo, thi):
                        o2 = o2p.tile([128, C], f32, tag="o2", name="o2")
                        nc.tensor.matmul(
                            o2[:], lhsT=swT[:, :, 128 * i:128 * (i + 1)],
                            rhs=OW[:], start=True, stop=True, perf_mode=DR)
                        nc.vector.scalar_tensor_tensor(
                            out=fx_full(i), in0=o2[:], scalar=ROI,
                            in1=fx_full(i), op0=ALU.mult, op1=ALU.add)
                        st6 = esb.tile([128, 6], f32, tag="st6")
                        nc.vector.bn_stats(out=st6[:], in_=fx_full(i))
                        nc.vector.bn_aggr(out=mv2[:, i, :], in_=st6[:])
                    rsqrt_stats(esb, mv2[:, tlo:thi, :], thi - tlo,
                                [(rstd2[:, tlo:thi], 1.0)])
                    for ci in range(4 * scc, 4 * (scc + 1)):
                        z2T = eio.tile([128, 2, 512], fp8, tag="z2T")
                        for j in range(4):
                            i = 4 * ci + j
                            z2 = esb.tile([128, C], bf16, tag="z2")
                            nc.gpsimd.tensor_tensor(
                                out=z2[:], in0=fx_full(i),
                                in1=rstd2[:, i:i + 1].broadcast_to([128, C]),
                                op=ALU.mult)
                            zt_ps = ztp2.tile([128, 256], bf16, tag="z2t")
                            for k in range(2):
                                nc.tensor.transpose(
                                    out=zt_ps[:, 128 * k:128 * (k + 1)],
                                    in_=z2[:, 128 * k:128 * (k + 1)],
                                    identity=id_bf[:])
                            nc.vector.tensor_copy(
                                out=z2T[:, :, 128 * j:128 * (j + 1)],
                                in_=zt_ps[:].rearrange("p (k t) -> p k t", k=2))
                        m1T = eio.tile([128, 8, 512], fp8, tag="m1T")
                        for mt in range(8):
                            mp = m1p.tile([128, 512], f32, tag="m1", name="m1")
                            nc.tensor.matmul(
                                mp[:],
                                lhsT=W['wm1'][:, :, 128 * mt:128 * (mt + 1)],
                                rhs=z2T[:], start=True, stop=True,
                                perf_mode=DR)
                            if not m1_bias:
                                nc.scalar.activation(
                                    out=m1T[:, mt, :], in_=mp[:],
                                    func=(AF.Identity if sim else AF.Gelu),
                                    scale=1.0 / SM1)
                            else:
                                nc.scalar.activation(
                                    out=m1T[:, mt, :], in_=mp[:],
                                    func=(AF.Identity if sim else AF.Gelu),
                                    scale=1.0 / SM1, bias=W['bm1c'][mt][:])
                        o_t = eio.tile([128, 4, C], f32, tag="ot")
                        for j in range(4):
                            i = 4 * ci + j
                            m2ps = smp.tile([128, C], f32, tag="m2", name="m2ps")
                            for kp in range(4):
                                nc.tensor.matmul(
                                    m2ps[:],
                                    lhsT=m1T[:, 2 * kp:2 * kp + 2,
                                             128 * j:128 * (j + 1)],
                                    rhs=wm2_4d[:, kp, :, :],
                                    start=(kp == 0), stop=(kp == 3),
                                    perf_mode=DR)
                            nc.vector.scalar_tensor_tensor(
                                out=o_t[:, j, :], in0=m2ps[:], scalar=1.0 / SM2,
                                in1=fx_full(i), op0=ALU.mult, op1=ALU.add)
                            if m2_bias:
                                nc.vector.tensor_add(out=o_t[:, j, :],
                                                     in0=o_t[:, j, :],
                                                     in1=W['bm2'][:])
                        nc.sync.dma_start(
                            out=out_d.ap()[:, 4 * ci:4 * (ci + 1), :],
                            in_=o_t[:])

    nc.compile()
    return nc


def _prep_inputs(NT, inputs):
    """Host-side weight folding + per-core input maps."""
    f = lambda x: np.asarray(x, np.float32)
    g1 = f(inputs["ln1_g"]); b1 = f(inputs["ln1_b"])
    g2 = f(inputs["ln2_g"]); b2 = f(inputs["ln2_b"])
    Wfx = f(inputs["Wfx"]); bfx = f(inputs["bfx"])
    Wx = f(inputs["Wx"]); bx = f(inputs["bx"])
    Wslice = f(inputs["Wslice"]); bslice = f(inputs["bslice"])
    temp = f(inputs["temperature"]).reshape(H)
    Wm1 = f(inputs["Wm1"]); bm1 = f(inputs["bm1"])
    Wm2 = f(inputs["Wm2"]); bm2 = f(inputs["bm2"])
    Wout = f(inputs["Wout"]); bout = f(inputs["bout"])
    Wq = f(inputs["Wq"]); Wk = f(inputs["Wk"]); Wv = f(inputs["Wv"])
    Wcq = f(inputs["Wcq"]); bcq = f(inputs["bcq"])
    Wck = f(inputs["Wck"]); bck = f(inputs["bck"])
    Wcv = f(inputs["Wcv"]); bcv = f(inputs["bcv"])
    scale = float(D) ** -0.5

    # block-diag Wslice scaled by 1/temperature
    Wbd = np.zeros((H * D, H * G), np.float32)
    for h in range(H):
        Wbd[h * D:(h + 1) * D, h * G:(h + 1) * G] = Wslice / temp[h]
    bslice_rep = np.concatenate([bslice / temp[h] for h in range(H)])

    # column-centered, fp8-scaled fused projection weights:
    # (x - mean(x)) @ W == x @ (W - colmean(W))
    Wfxp = g1[:, None] * Wfx
    Wxs = (g1[:, None] * Wx) @ Wbd
    Wfxp_c = (Wfxp - Wfxp.mean(0, keepdims=True)) * SA
    Wxs_c = (Wxs - Wxs.mean(0, keepdims=True)) * SB
    wall_full = np.concatenate([Wfxp_c, Wxs_c], axis=1)       # [C, 512]
    wall_dr = np.ascontiguousarray(
        wall_full.reshape(2, 128, 512).transpose(1, 0, 2)).astype(F8)
    ball_f = np.concatenate([(b1 @ Wfx + bfx) * SA,
                             ((b1 @ Wx + bx) @ Wbd + bslice_rep) * SB])
    ball = ball_f[None, :].astype(BF)
    proj_bias = bool(np.any(ball_f != 0.0))

    # MLP weights: column-centered g2*Wm1, fp8 DR layouts
    W1 = g2[:, None] * Wm1
    W1_c = (W1 - W1.mean(0, keepdims=True)) * SM1              # [C, 4C]
    wm1_dr = np.ascontiguousarray(
        W1_c.reshape(2, 128, 4 * C).transpose(1, 0, 2)).astype(F8)
    bm1p = (b2 @ Wm1 + bm1).astype(np.float32)
    m1_bias = bool(np.any(bm1p != 0.0))
    wm2_dr = np.ascontiguousarray(
        (Wm2 * SM2).reshape(4, 2, 128, C).transpose(2, 0, 1, 3)
        .reshape(128, 8 * C)).astype(F8)
    bm2_rep = np.ascontiguousarray(
        np.broadcast_to(bm2[None, :], (128, C)), np.float32)
    m2_bias = bool(np.any(bm2 != 0.0))

    # Wout folded into attention values (scaled by SOW for fp8 range)
    Wvw = np.stack([Wv @ Wout[32 * h:32 * (h + 1), :] for h in range(H)],
                   1) * SOW
    Wcvw = np.stack([Wcv @ Wout[32 * h:32 * (h + 1), :] for h in range(H)],
                    1) * SOW
    bcvw = np.stack([bcv @ Wout[32 * h:32 * (h + 1), :] for h in range(H)],
                    0) * SOW
    cv_bias = bool(np.any(bcv != 0.0))
    bout8 = np.ascontiguousarray(
        np.broadcast_to(bout[None, None, :] * (SOW / H), (128, 2, C)),
        np.float32)
    wout_bias = bool(np.any(bout != 0.0))

    mwv = float(1.0 / (1.0 + np.exp(-f(inputs["state_mixing"]))))
    mw = np.full((128, 1), mwv, np.float32)
    omw = np.full((128, 1), 1.0 - mwv, np.float32)

    id_f = np.eye(128, dtype=np.float32)

    common = dict(
        wall=wall_dr, ball=ball, wm1=wm1_dr, bm1=bm1p, wm2=wm2_dr,
        bm2=bm2_rep, bout8=bout8,
        wq=Wq, wk=Wk, wcq=Wcq, bcq=(bcq * scale).astype(np.float32),
        wck=Wck, bck=bck,
        wvw=np.ascontiguousarray(Wvw, np.float32),
        wcvw=np.ascontiguousarray(Wcvw, np.float32),
        bcvw=np.ascontiguousarray(bcvw[None, :, :], np.float32),
        mw=mw, omw=omw,
        id_bf=id_f.astype(BF), id_f8=id_f.astype(F8), id_f=id_f,
        ones_bf=np.ones((1, 128), BF), ones64=np.ones((1, 64), np.float32),
    )

    fx = f(inputs["fx"])
    ctxt = f(inputs["context"])
    in_maps = []
    for core in range(NCORES):
        b, s = core // CPB, core % CPB
        x = fx[b, s * NT:(s + 1) * NT, :]                      # [NT, C]
        m = dict(common)
        m["fxq"] = np.ascontiguousarray(
            x.reshape(NT // 128, 128, C).transpose(1, 0, 2))
        m["fxT"] = np.ascontiguousarray(
            x.T.reshape(2, 128, NT).transpose(1, 0, 2)).astype(F8)
        m["ctx"] = np.ascontiguousarray(ctxt[b])
        in_maps.append(m)
    return in_maps, (proj_bias, m1_bias, wout_bias, m2_bias, cv_bias)


_CACHE = {}


def _get_compiled(NT, flags):
    key = (NT,) + flags
    if key not in _CACHE:
        _CACHE[key] = _build(NT, flags)
    return _CACHE[key]


def kernel(**inputs):
    from concourse.bass_utils import run_bass_kernel_spmd
    NT = NT_FULL
    in_maps, flags = _prep_inputs(NT, inputs)
    nc = _get_compiled(NT, flags)
    res = run_bass_kernel_spmd(nc, in_maps, list(range(NCORES)))
    out = np.empty((B, N, C), np.float32)
    for core in range(NCORES):
        b, s = core // CPB, core % CPB
        out[b, s * NT:(s + 1) * NT, :] = (
            res.results[core]["out"].transpose(1, 0, 2).reshape(NT, C))
    return out
